# revision 29
# baseline (speedup 1.0000x reference)
"""Trainium2 Bass kernel for 16-head MultiHeadAttention (EMB=1024, seq=2048, batch=2).

Sharding: 8 cores = 2 batches x 4 head-groups (4 heads each).
Per core: Q/K/V projections with column-sharded weights, attention over its
4 heads, and per-head-pair partial output projections with the row-sharded
Wo.  The host sums the 8 partials per batch and adds the bv/bo terms.

Schedule: the kernel is a single software pipeline over 64 "kp" steps
(8 units of (q-chunk, head-pair) x 8 key-pair steps each).  ScalarE's exp
(~147us total) is the hard floor; every projection / output matmul is
injected as filler between attention matmuls so TensorE work (~137us)
hides completely under it.  Emission order == per-engine FIFO order.
"""

import sys

for _p in ("/opt/trn_rl_repo", "/root/.axon_site/_ro/trn_rl_repo"):
    if _p not in sys.path:
        sys.path.insert(0, _p)

import numpy as np
import ml_dtypes

BF16 = ml_dtypes.bfloat16

N = 2048          # sequence length
E = 1024          # embedding
HDL = 256         # local head width per core (4 heads x 64)
D = 64            # head dim
NHL = 4           # local heads
EC = 8            # e-chunks of 128
NT = 16           # n-tiles of 128
SCALE = 1.0 / 32.0  # 1/sqrt(E)

UNITS = [(0, 0), (1, 0), (2, 0), (3, 0), (0, 1), (1, 1), (2, 1), (3, 1)]
PV_LAG = 4
WARM_MMS = 56
TRICKLE_WARM = 6

MAX_DRAIN_WAITS = 1

_compiled = {}


def _patch_drain(tile_mod, mybir):
    """Walrus in this container rejects >1 sync wait on the final Drain;
    spread the end-of-kernel waits over nop instructions instead."""
    from concourse.vector_clock import ScopedClock

    def _drain_and_barrier(self, tick_clock, wait_clock):
        nc = self.nc
        probe = nc.sync.nop(nofuse=True)
        wait_clock.add_sem_waits(probe.ins, ScopedClock({None: tick_clock.global_clock}))
        si = probe.ins.sync_info
        waits = list(si.on_wait) if si is not None and si.on_wait else []
        if len(waits) > MAX_DRAIN_WAITS:
            si.on_wait = waits[:MAX_DRAIN_WAITS]
            rest = waits[MAX_DRAIN_WAITS:]
            for i in range(0, len(rest), MAX_DRAIN_WAITS):
                nop = nc.sync.nop(nofuse=True)
                nsi = nop.ins.sync_info
                chunk = rest[i : i + MAX_DRAIN_WAITS]
                if nsi is None:
                    nop.ins.sync_info = mybir.SyncInfo(on_wait=chunk, on_update=[])
                else:
                    nsi.on_wait = chunk
        nc.sync.drain()
        nc.all_engine_barrier()
        assert self.sems is not None
        popped = nc._tile_sem_poison_stack.pop()
        assert popped is self._sem_poison
        nc.clear_and_free_semaphores(list(self.sems.allocated().values()))
        nc.all_engine_barrier()

    tile_mod.TileContext._drain_and_barrier = _drain_and_barrier


def _split_excess_waits(nc, mybir):
    """This container's walrus rejects >1 sync wait per instruction.  Move
    extra waits onto same-engine NOPs inserted right before the instruction
    (engine streams execute in block order, so semantics are unchanged)."""
    n = 0
    for fn in nc.m.functions:
        for bb in fn.blocks:
            out = []
            for inst in bb.instructions:
                si = inst.sync_info
                if si is not None and si.on_wait and len(si.on_wait) > 1:
                    waits = list(si.on_wait)
                    si.on_wait = waits[-1:]
                    for w in waits[:-1]:
                        n += 1
                        nop = mybir.InstNoOp(
                            name=f"I-waitsplit-{n}",
                            engine=inst.engine,
                            sync_info=mybir.SyncInfo(on_wait=[w], on_update=[]),
                            text_hint="waitsplit",
                            bass_nofuse=True,
                        )
                        out.append(nop)
                out.append(inst)
            if n:
                bb.instructions = out


def _build():
    import concourse.bass as bass
    import concourse.mybir as mybir
    import concourse.tile as tile

    _patch_drain(tile, mybir)

    bf = mybir.dt.bfloat16
    f32 = mybir.dt.float32

    nc = bass.Bass()
    xa_d = nc.dram_tensor("xTa", [EC, 128, 512], bf, kind="ExternalInput")
    xb1_d = nc.dram_tensor("xTb1", [EC, 128, 512], bf, kind="ExternalInput")
    xb2_d = nc.dram_tensor("xTb2", [EC, 128, 1024], bf, kind="ExternalInput")
    # weights pre-laid host-side in partition-major order so every DMA moves
    # long contiguous lines (2-4KB) instead of 256B strided scatters
    wq_d = nc.dram_tensor("wqT", [128, 2, EC, 128], bf, kind="ExternalInput")
    wk_d = nc.dram_tensor("wkT", [128, 2, EC, 128], bf, kind="ExternalInput")
    wv_d = nc.dram_tensor("wvT", [128, EC, HDL], bf, kind="ExternalInput")
    wo_d = nc.dram_tensor("woT", [128, 2, E], bf, kind="ExternalInput")
    bq_d = nc.dram_tensor("bqs", [128, 2, 1], f32, kind="ExternalInput")
    bk_d = nc.dram_tensor("bks", [128, 2, 1], f32, kind="ExternalInput")
    y_d = [
        nc.dram_tensor("y0", [N, E], bf, kind="ExternalOutput"),
        nc.dram_tensor("y1", [N, E], bf, kind="ExternalOutput"),
    ]

    with tile.TileContext(nc) as tc:
        _emit(nc, tc, tile, mybir, xa_d, xb1_d, xb2_d, wq_d, wk_d, wv_d, wo_d, bq_d, bk_d, y_d)
    _split_excess_waits(nc, mybir)
    return nc


def _emit(nc, tc, tile, mybir, xa_d, xb1_d, xb2_d, wq_d, wk_d, wv_d, wo_d, bq_d, bk_d, y_d):
    import concourse.bass as bass
    from contextlib import ExitStack

    bf = mybir.dt.bfloat16
    f32 = mybir.dt.float32
    Exp = mybir.ActivationFunctionType.Exp

    ctx = ExitStack()
    with ctx:
        persist = ctx.enter_context(tc.tile_pool(name="persist", bufs=1))
        # PSUM budget (8 banks): en 2x2 + pv 2x1 + filler 2x1
        psen = ctx.enter_context(tc.tile_pool(name="psen", bufs=2, space="PSUM"))
        pvp = ctx.enter_context(tc.tile_pool(name="pvp", bufs=2, space="PSUM"))
        fillp = ctx.enter_context(tc.tile_pool(name="fillp", bufs=2, space="PSUM"))
        attp = ctx.enter_context(tc.tile_pool(name="attp", bufs=22))
        normp = ctx.enter_context(tc.tile_pool(name="normp", bufs=4))
        pvdp = ctx.enter_context(tc.tile_pool(name="pvdp", bufs=4))
        stagep = ctx.enter_context(tc.tile_pool(name="stagep", bufs=4))
        dramp = ctx.enter_context(tc.tile_pool(name="dramp", bufs=4, space="DRAM"))

        # ---- persistent SBUF ----
        x_sb = persist.tile([128, EC, N], bf)
        wq_sb = persist.tile([128, 2, EC, 128], bf)
        wk_sb = persist.tile([128, 2, EC, 128], bf)
        wv_sb = persist.tile([128, EC, HDL], bf)
        wo_sb = persist.tile([128, 2, E], bf)
        bq_sb = persist.tile([128, 2, 1], f32)
        bk_sb = persist.tile([128, 2, 1], f32)
        qT_sb = persist.tile([128, 2, N], bf)
        kT_sb = persist.tile([128, 2, N], bf)
        # V with per-head aug column: [V(0:64) | ones(64) | pad]
        v_sb = persist.tile([128, NT, NHL, 66], bf)
        outn_sb = persist.tile([128, 2, N], bf)
        warm_sb = persist.tile([128, 64], bf)
        warmf_in = persist.tile([1, 8], f32)
        warmf_out = persist.tile([1, 8], f32)
        ones_sb = persist.tile([1, 64], bf)

        # ---- t0: PE warm-up + exp table preload (run during input DMA) ----
        nc.vector.memset(warm_sb[:, :], 0.0)
        nc.vector.memset(warmf_in[:, :], 0.0)
        nc.vector.memset(v_sb[:, :, :, 64:65], 1.0)
        nc.vector.memset(ones_sb[:, :], 1.0)
        wacc = fillp.tile([128, 512], f32, tag="acc", name="warmacc")
        for i in range(WARM_MMS):
            nc.tensor.matmul(
                wacc[0:64, 0:64],
                lhsT=warm_sb[:, 0:64],
                rhs=warm_sb[:, 0:64],
                start=True,
                stop=True,
            )

        # ---- input DMAs, priority-ordered ----
        # weights move contiguous 2-4KB lines; the startup-critical x chunk
        # (first 512 query columns) fans out over FOUR rings so the first
        # q/k projections are fed ~2x sooner
        nc.sync.dma_start(out=wq_sb[:, 0, :, :], in_=wq_d[:, 0, :, :])
        nc.gpsimd.dma_start(out=wk_sb[:, 0, :, :], in_=wk_d[:, 0, :, :])
        nc.gpsimd.dma_start(out=bq_sb[:, :, :], in_=bq_d[:, :, :])
        nc.gpsimd.dma_start(out=bk_sb[:, :, :], in_=bk_d[:, :, :])
        xa_rings = [nc.sync, nc.gpsimd, nc.scalar]
        for ec in range(EC):
            xa_rings[ec % 3].dma_start(out=x_sb[:, ec, 0:512], in_=xa_d[ec, :, :])
        for ec in range(EC):
            e = nc.scalar if ec % 2 == 0 else nc.gpsimd
            e.dma_start(out=x_sb[:, ec, 512:1024], in_=xb1_d[ec, :, :])
        # exp table preload rides the scalar queue after its input doorbells
        nc.scalar.activation(warmf_out[:, :], warmf_in[:, :], Exp)
        nc.sync.dma_start(out=wv_sb[:, :, :], in_=wv_d[:, :, :])
        for ec in range(EC):
            e = nc.sync if ec % 2 == 0 else nc.gpsimd
            e.dma_start(out=x_sb[:, ec, 1024:2048], in_=xb2_d[ec, :, :])
        nc.gpsimd.dma_start(out=wk_sb[:, 1, :, :], in_=wk_d[:, 1, :, :])
        nc.gpsimd.dma_start(out=wq_sb[:, 1, :, :], in_=wq_d[:, 1, :, :])
        nc.sync.dma_start(out=wo_sb[:, :, :], in_=wo_d[:, :, :])

        # ---- filler group emitters (emitted in 4-MM halves so a group
        # never delays the next en by more than ~0.9us in the PE FIFO) ----
        half_state = {}

        def emit_qk_half(mat, hc, qc, part):
            dst, w_sb, b_sb = (
                (qT_sb, wq_sb, bq_sb) if mat == "q" else (kT_sb, wk_sb, bk_sb)
            )
            nsl = slice(qc * 512, (qc + 1) * 512)
            if part == 0:
                acc = fillp.tile([128, 512], f32, tag="acc", name=f"{mat}acc{hc}_{qc}")
                half_state[(mat, hc, qc)] = acc
            acc = half_state[(mat, hc, qc)]
            for ec in range(part * 4, part * 4 + 4):
                nc.tensor.matmul(
                    acc[:, :],
                    lhsT=w_sb[:, hc, ec, :],
                    rhs=x_sb[:, ec, nsl],
                    start=(ec == 0),
                    stop=(ec == EC - 1),
                )
            if part == 1:
                del half_state[(mat, hc, qc)]
                nc.vector.tensor_scalar_add(dst[:, hc, nsl], acc[:, :], b_sb[:, hc, :])

        def emit_v_half(nt, part):
            if part == 0:
                acc = fillp.tile([128, 512], f32, tag="acc", name=f"vacc{nt}")
                half_state[("v", nt)] = acc
            acc = half_state[("v", nt)]
            vacc = acc[:, 0:HDL]
            for ec in range(part * 4, part * 4 + 4):
                nc.tensor.matmul(
                    vacc,
                    lhsT=x_sb[:, ec, nt * 128 : (nt + 1) * 128],
                    rhs=wv_sb[:, ec, :],
                    start=(ec == 0),
                    stop=(ec == EC - 1),
                )
            if part == 1:
                del half_state[("v", nt)]
                nc.vector.tensor_copy(
                    out=v_sb[:, nt, :, 0:64],
                    in_=acc[:, 0:HDL].rearrange("p (h d) -> p h d", d=64),
                )

        def emit_qk_group(mat, hc, qc):
            emit_qk_half(mat, hc, qc, 0)
            emit_qk_half(mat, hc, qc, 1)

        def emit_v_group(nt):
            emit_v_half(nt, 0)
            emit_v_half(nt, 1)

        ystage_cur = {}

        def emit_out_atom(qs, hc, k, copy_eng=None, deep_psum=False):
            # one (nt, ech) micro-step of the output projection; spread one
            # per slot so the PSUM-bank recycle (via the copy) never blocks
            # the PE FIFO
            nt = qs * 4 + k // 2
            ech = k % 2
            if ech == 0:
                ystage_cur[(qs, hc)] = stagep.tile(
                    [128, E], bf, tag="ystage", name=f"ystage{nt}_{hc}"
                )
            ystage = ystage_cur[(qs, hc)]
            esl = slice(ech * 512, (ech + 1) * 512)
            if deep_psum and k % 2 == 1:
                # at the tail the en pool's banks are free: alternating pools
                # gives a 4-bank rotation so the MM->copy chain never blocks
                ent = psen.tile([128, 2, 512], f32, tag="en", name=f"oaccp{nt}_{hc}_{ech}")
                acc = ent[:, 0, :]
            else:
                acc = fillp.tile([128, 512], f32, tag="acc", name=f"oacc{nt}_{hc}_{ech}")
            nc.tensor.matmul(
                acc[:, :],
                lhsT=outn_sb[:, hc, nt * 128 : (nt + 1) * 128],
                rhs=wo_sb[:, hc, esl],
                start=True,
                stop=True,
            )
            eng = copy_eng or nc.vector
            if eng is nc.scalar:
                nc.scalar.copy(out=ystage[:, esl], in_=acc[:, :])
            else:
                eng.tensor_copy(out=ystage[:, esl], in_=acc[:, :])
            if ech == 1:
                nc.sync.dma_start(
                    out=y_d[hc][nt * 128 : (nt + 1) * 128, :], in_=ystage[:, :]
                )

        # ---- attention pipeline state ----
        att_ring = {}
        pv_tiles = {}

        def emit_en(g):
            u, kp = divmod(g, 8)
            qs, hc = UNITS[u]
            nsl = slice(qs * 512, (qs + 1) * 512)
            for h in (0, 1):
                en = psen.tile([128, 2, 512], f32, tag="en", name=f"en{g}_{h}")
                dsl = slice(h * 64, (h + 1) * 64)
                for j in (0, 1):
                    kt = 2 * kp + j
                    nc.tensor.matmul(
                        en[:, j, :],
                        lhsT=kT_sb[dsl, hc, kt * 128 : (kt + 1) * 128],
                        rhs=qT_sb[dsl, hc, nsl],
                        start=True,
                        stop=True,
                        tile_position=(h * 64, 0),
                    )
                a = attp.tile([128, 2, 512], bf, tag="att", name=f"att{g}_{h}")
                nc.scalar.activation(a[:, :, :], en[:, :, :], Exp)
                att_ring[(g, h)] = a

        def emit_pv(g):
            u, kp = divmod(g, 8)
            qs, hc = UNITS[u]
            if kp == 0:
                pv_tiles[u] = [
                    pvp.tile([128, 512], f32, tag="pv", name=f"pv{u}_{h}")
                    for h in (0, 1)
                ]
            pv = pv_tiles[u]
            for h in (0, 1):
                a = att_ring.pop((g, h))
                for j in (0, 1):
                    kt = 2 * kp + j
                    nc.tensor.matmul(
                        pv[h][0:65, :],
                        lhsT=v_sb[:, kt, hc * 2 + h, 0:65],
                        rhs=a[:, j, :],
                        start=(kp == 0 and j == 0),
                        stop=(kp == 7 and j == 1),
                    )

        pvd_tiles = {}
        rdram_tiles = {}

        def emit_norm_a(u, dq=None, chain=True):
            # drain pv PSUM -> SBUF (frees the banks for the next unit), then
            # kick off the transposed-reciprocal DMA chain
            dq = dq or nc.gpsimd
            pv = pv_tiles.pop(u)
            pvd = [pvdp.tile([128, 512], f32, tag="pvd", name=f"pvd{u}_{h}") for h in (0, 1)]
            for h in (0, 1):
                nc.vector.tensor_copy(out=pvd[h][0:65, :], in_=pv[h][0:65, :])
            pvd_tiles[u] = pvd
            if not chain:
                return
            sdram = dramp.tile([1, 1024], f32, tag="sdram")
            for h in (0, 1):
                dq.dma_start(
                    out=sdram[0:1, h * 512 : (h + 1) * 512], in_=pvd[h][64:65, :]
                )
            wide = bass.AP(
                tensor=sdram.tensor, offset=sdram.offset, ap=[[32, 32], [1, 32]]
            )
            sw = normp.tile([32, 32], f32, tag="sw")
            dq.dma_start(out=sw[0:32, :], in_=wide)
            rw = normp.tile([32, 32], f32, tag="rw")
            nc.vector.reciprocal(rw[0:32, :], sw[0:32, :])
            rdram = dramp.tile([1, 1024], f32, tag="rdram")
            wide_r = bass.AP(
                tensor=rdram.tensor, offset=rdram.offset, ap=[[32, 32], [1, 32]]
            )
            dq.dma_start(out=wide_r, in_=rw[0:32, :])
            rdram_tiles[u] = rdram

        def emit_norm_b(u, dq=None):
            qs, hc = UNITS[u]
            nsl = slice(qs * 512, (qs + 1) * 512)
            dq = dq or nc.gpsimd
            pvd = pvd_tiles.pop(u)
            rdram = rdram_tiles.pop(u)
            bcast = normp.tile([64, 1024], f32, tag="bcast")
            bsrc = bass.AP(
                tensor=rdram.tensor, offset=rdram.offset, ap=[[0, 64], [1, 1024]]
            )
            dq.dma_start(out=bcast[0:64, :], in_=bsrc)
            nc.vector.tensor_mul(
                outn_sb[0:64, hc, nsl], pvd[0][0:64, :], bcast[0:64, 0:512]
            )
            ost = normp.tile([64, 512], bf, tag="ost")
            nc.vector.tensor_mul(ost[0:64, :], pvd[1][0:64, :], bcast[0:64, 512:1024])
            nc.sync.dma_start(out=outn_sb[64:128, hc, nsl], in_=ost[0:64, :])

        def emit_norm_b7(u):
            # tail-unit normalization with no DRAM round trips: reciprocal of
            # the aug row on DVE, partition-broadcast via a rank-1 PE matmul
            qs, hc = UNITS[u]
            nsl = slice(qs * 512, (qs + 1) * 512)
            pvd = pvd_tiles.pop(u)
            rw = normp.tile([1, 2, 512], bf, tag="rw7")
            with nc.allow_low_precision(reason="bf16 denom reciprocal, 0.4% rel"):
                for h in (0, 1):
                    nc.vector.reciprocal(rw[0:1, h, :], pvd[h][64:65, :])
            bc = psen.tile([128, 2, 512], f32, tag="en", name="bcast7")
            for h in (0, 1):
                nc.tensor.matmul(
                    bc[0:64, h, :],
                    lhsT=ones_sb[0:1, 0:64],
                    rhs=rw[0:1, h, :],
                    start=True,
                    stop=True,
                )
            nc.vector.tensor_mul(outn_sb[0:64, hc, nsl], pvd[0][0:64, :], bc[0:64, 0, :])
            ost = normp.tile([64, 512], bf, tag="ost")
            nc.vector.tensor_mul(ost[0:64, :], pvd[1][0:64, :], bc[0:64, 1, :])
            nc.gpsimd.dma_start(out=outn_sb[64:128, hc, nsl], in_=ost[0:64, :])

        # ---- precomputed per-slot action table ----
        # pv lag: deep (6) during U0 to shed front-loaded filler pressure,
        # 4 at steady state, tapered at the end to shrink the tail.
        def pv_slot(g):
            if g < 8:
                return g + 6
            if g < 54:
                return g + 4
            if g < 58:
                return g + 3
            return min(g + 2, 64)  # taper the lag so the tail drains fast

        PV_AT = {}
        for g in range(64):
            PV_AT.setdefault(min(pv_slot(g), 64), []).append(g)

        norm_a_slot = {}
        for u in range(8):
            norm_a_slot[u] = min(pv_slot(8 * u + 7), 64)
        norm_b_slot = {u: norm_a_slot[u] + 2 for u in range(8)}

        SLOTS = {
            1: [("k", 0, 1)],
            3: [("k", 0, 2)],
            4: [("v", 0), ("v", 1)],
            5: [("k", 0, 3), ("v", 2)],
            6: [("q", 0, 1), ("v", 3), ("v", 4)],
            7: [("v", 5), ("v", 6)],
            8: [("v", 7), ("v", 8)],
            9: [("v", 9), ("v", 10)],
            10: [("v", 11), ("q", 0, 2)],
            11: [("v", 12), ("v", 13)],
            12: [("v", 14), ("v", 15)],
            18: [("q", 0, 3)],
            24: [("q", 1, 0)],
            26: [("k", 1, 0)],
            30: [("k", 1, 1)],
            32: [("k", 1, 2)],
            34: [("k", 1, 3)],
            36: [("q", 1, 1)],
            44: [("q", 1, 2)],
            52: [("q", 1, 3)],
        }
        for u in range(8):
            for k in range(8):
                # unit 6's atoms pack 3-per-slot so they all land inside the
                # taper window, filling the PE while the last exps drain
                step = k // 3 if u == 6 else k
                s = norm_b_slot[u] + 1 + step
                if s < 64:
                    SLOTS.setdefault(s, []).append(("oa", UNITS[u][0], UNITS[u][1], k))

        # ---- preamble compute ----
        # trickle warm matmuls between the projection halves keep the PE HAM
        # from declocking across input-DMA hiccups (targets the pv pool,
        # which has no allocations until pv(0) several slots later)
        wacc_t = pvp.tile([128, 512], f32, tag="pv", name="warmtrickle")

        def warm_trickle(n):
            for _ in range(n):
                nc.tensor.matmul(
                    wacc_t[0:64, 0:64],
                    lhsT=warm_sb[:, 0:64],
                    rhs=warm_sb[:, 0:64],
                    start=True,
                    stop=True,
                )

        emit_qk_half("q", 0, 0, 0)
        warm_trickle(TRICKLE_WARM)
        emit_qk_half("k", 0, 0, 0)
        warm_trickle(TRICKLE_WARM)
        emit_qk_half("q", 0, 0, 1)
        warm_trickle(TRICKLE_WARM)
        emit_qk_half("k", 0, 0, 1)

        # ---- main pipeline ----
        emitted_atoms = set()

        def do_slot(g):
            halves = []
            for f in SLOTS.get(g, []):
                if f[0] in ("q", "k", "v"):
                    halves.append((f, 0))
                    halves.append((f, 1))
                else:
                    halves.append((f, None))

            def emit_half(item):
                f, part = item
                if f[0] in ("q", "k"):
                    emit_qk_half(*f, part)
                elif f[0] == "v":
                    emit_v_half(f[1], part)
                else:
                    emit_out_atom(*f[1:])
                    emitted_atoms.add(f[1:])

            # front-load one filler half ahead of the pv/norm section, but
            # never an out-atom: those may wait on a norm DMA and would
            # stall the in-order PE FIFO ahead of the pv matmuls
            if halves and halves[0][0][0] != "oa":
                emit_half(halves.pop(0))
            for gp in PV_AT.get(g, []):
                emit_pv(gp)
                if gp % 8 == 7 and gp // 8 in norm_a_slot:
                    # unit 7 skips the DRAM reciprocal dance; its norm runs
                    # through a PE broadcast at the tail instead
                    emit_norm_a(gp // 8, chain=(gp // 8 != 7))
                    del norm_a_slot[gp // 8]
            for u in range(7):
                if norm_b_slot.get(u) == g:
                    emit_norm_b(u)
            for item in halves:
                emit_half(item)

        for g in range(64):
            emit_en(g)
            do_slot(g)

        # ---- tail drain ----
        do_slot(64)
        # U6 atoms that didn't fit run while norm(7)'s reciprocal is in flight
        for k in range(8):
            if (2, 1, k) not in emitted_atoms:
                emit_out_atom(
                    2, 1, k, copy_eng=nc.scalar if k % 2 else nc.vector, deep_psum=True
                )
        # short HAM-warm bridge over the norm(7) reciprocal latency
        wacc2 = fillp.tile([128, 512], f32, tag="acc", name="warmacc2")
        for i in range(16):
            nc.tensor.matmul(
                wacc2[0:64, 0:256],
                lhsT=warm_sb[:, 0:64],
                rhs=x_sb[:, 0, 0:256],
                start=True,
                stop=True,
            )
        emit_norm_b7(7)
        # final unit's projection; per 128-row tile the two e-halves run as
        # FD=512 matmuls (fp32 PSUM out caps FD at one bank) with copies
        # alternating scalar/vector, and the y DMAs split across both rings
        for nt in range(12, 16):
            acc = psen.tile([128, 2, 512], f32, tag="en", name=f"oacc7_{nt}")
            ystage = stagep.tile([128, E], bf, tag="ystage", name=f"ystage7_{nt}")
            nsl_r = slice(nt * 128, (nt + 1) * 128)
            for ech in (0, 1):
                esl = slice(ech * 512, (ech + 1) * 512)
                nc.tensor.matmul(
                    acc[:, ech, :],
                    lhsT=outn_sb[:, 1, nsl_r],
                    rhs=wo_sb[:, 1, esl],
                    start=True,
                    stop=True,
                )
                if ech:
                    nc.scalar.copy(out=ystage[:, esl], in_=acc[:, ech, :])
                    nc.gpsimd.dma_start(out=y_d[1][nsl_r, esl], in_=ystage[:, esl])
                else:
                    nc.vector.tensor_copy(out=ystage[:, esl], in_=acc[:, ech, :])
                    nc.sync.dma_start(out=y_d[1][nsl_r, esl], in_=ystage[:, esl])


def _prep_core_inputs(x, Wq, bq, Wk, bk, Wv, bv, Wo, bo):
    """Build the 8 per-core input maps (host-side sharding + layout)."""
    xT_by_batch = []
    for b in range(2):
        xT = np.ascontiguousarray(x[b].T).astype(BF16).reshape(EC, 128, N)
        xa = np.ascontiguousarray(xT[:, :, 0:512])
        xb1 = np.ascontiguousarray(xT[:, :, 512:1024])
        xb2 = np.ascontiguousarray(xT[:, :, 1024:2048])
        xT_by_batch.append((xa, xb1, xb2))
    in_maps = []
    for c in range(8):
        b, g = divmod(c, 4)
        hsl = slice(g * HDL, (g + 1) * HDL)

        def pair_major(wT):
            # wT: [E, HDL] -> [128, 2, EC, 128] (partition-major, so every
            # DMA line is a contiguous 2KB run per partition)
            return np.ascontiguousarray(
                wT.reshape(EC, 128, 2, 128).transpose(1, 2, 0, 3)
            )

        wqT = pair_major((Wq[hsl, :] * SCALE).T.astype(BF16))
        wkT = pair_major(Wk[hsl, :].T.astype(BF16))
        wvT = np.ascontiguousarray(
            Wv[hsl, :].T.astype(BF16).reshape(EC, 128, HDL).transpose(1, 0, 2)
        )
        woT = np.ascontiguousarray(
            Wo[:, hsl].T.astype(BF16).reshape(2, 128, E).transpose(1, 0, 2)
        )
        bqs = np.ascontiguousarray(
            (bq[hsl] * SCALE).astype(np.float32).reshape(2, 128).T.reshape(128, 2, 1)
        )
        bks = np.ascontiguousarray(
            bk[hsl].astype(np.float32).reshape(2, 128).T.reshape(128, 2, 1)
        )
        in_maps.append(
            {
                "xTa": xT_by_batch[b][0],
                "xTb1": xT_by_batch[b][1],
                "xTb2": xT_by_batch[b][2],
                "wqT": wqT,
                "wkT": wkT,
                "wvT": wvT,
                "woT": woT,
                "bqs": bqs,
                "bks": bks,
            }
        )
    return in_maps


def run(inputs, trace=False, trace_kwargs=None):
    """Compile (cached), execute on 8 cores, gather.  Returns (y, results)."""
    from concourse.bass_utils import run_bass_kernel_spmd

    if "nc" not in _compiled:
        _compiled["nc"] = _build()
    nc = _compiled["nc"]

    in_maps = _prep_core_inputs(**inputs)
    kwargs = {}
    if trace:
        kwargs["trace"] = True
        kwargs["trace_kwargs"] = trace_kwargs or {}
    res = run_bass_kernel_spmd(nc, in_maps, core_ids=list(range(8)), **kwargs)

    x, Wo, bo, bv = inputs["x"], inputs["Wo"], inputs["bo"], inputs["bv"]
    y = np.zeros((2, N, E), np.float32)
    for c in range(8):
        b = c // 4
        y[b] += res.results[c]["y0"].astype(np.float32)
        y[b] += res.results[c]["y1"].astype(np.float32)
    y += (np.asarray(bv, np.float32) @ np.asarray(Wo, np.float32).T + np.asarray(bo, np.float32))[None, None, :]
    return y.astype(np.float32), res


def kernel(**inputs):
    inputs = {k: np.asarray(v) for k, v in inputs.items()}
    y, _ = run(inputs)
    return y



# revision 30
# speedup vs baseline: 1.0109x; 1.0109x over previous
"""Trainium2 Bass kernel for 16-head MultiHeadAttention (EMB=1024, seq=2048, batch=2).

Sharding: 8 cores = 2 batches x 4 head-groups (4 heads each).
Per core: Q/K/V projections with column-sharded weights, attention over its
4 heads, and per-head-pair partial output projections with the row-sharded
Wo.  The host sums the 8 partials per batch and adds the bv/bo terms.

Schedule: the kernel is a single software pipeline over 64 "kp" steps
(8 units of (q-chunk, head-pair) x 8 key-pair steps each).  ScalarE's exp
(~147us total) is the hard floor; every projection / output matmul is
injected as filler between attention matmuls so TensorE work (~137us)
hides completely under it.  Emission order == per-engine FIFO order.
"""

import sys

for _p in ("/opt/trn_rl_repo", "/root/.axon_site/_ro/trn_rl_repo"):
    if _p not in sys.path:
        sys.path.insert(0, _p)

import numpy as np
import ml_dtypes

BF16 = ml_dtypes.bfloat16

N = 2048          # sequence length
E = 1024          # embedding
HDL = 256         # local head width per core (4 heads x 64)
D = 64            # head dim
NHL = 4           # local heads
EC = 8            # e-chunks of 128
NT = 16           # n-tiles of 128
SCALE = 1.0 / 32.0  # 1/sqrt(E)

UNITS = [(0, 0), (1, 0), (2, 0), (3, 0), (0, 1), (1, 1), (2, 1), (3, 1)]
PV_LAG = 4
WARM_MMS = 56
TRICKLE_WARM = 6

MAX_DRAIN_WAITS = 1

_compiled = {}


def _patch_drain(tile_mod, mybir):
    """Walrus in this container rejects >1 sync wait on the final Drain;
    spread the end-of-kernel waits over nop instructions instead."""
    from concourse.vector_clock import ScopedClock

    def _drain_and_barrier(self, tick_clock, wait_clock):
        nc = self.nc
        probe = nc.sync.nop(nofuse=True)
        wait_clock.add_sem_waits(probe.ins, ScopedClock({None: tick_clock.global_clock}))
        si = probe.ins.sync_info
        waits = list(si.on_wait) if si is not None and si.on_wait else []
        if len(waits) > MAX_DRAIN_WAITS:
            si.on_wait = waits[:MAX_DRAIN_WAITS]
            rest = waits[MAX_DRAIN_WAITS:]
            for i in range(0, len(rest), MAX_DRAIN_WAITS):
                nop = nc.sync.nop(nofuse=True)
                nsi = nop.ins.sync_info
                chunk = rest[i : i + MAX_DRAIN_WAITS]
                if nsi is None:
                    nop.ins.sync_info = mybir.SyncInfo(on_wait=chunk, on_update=[])
                else:
                    nsi.on_wait = chunk
        nc.sync.drain()
        nc.all_engine_barrier()
        assert self.sems is not None
        popped = nc._tile_sem_poison_stack.pop()
        assert popped is self._sem_poison
        nc.clear_and_free_semaphores(list(self.sems.allocated().values()))
        nc.all_engine_barrier()

    tile_mod.TileContext._drain_and_barrier = _drain_and_barrier


def _split_excess_waits(nc, mybir):
    """This container's walrus rejects >1 sync wait per instruction.  Move
    extra waits onto same-engine NOPs inserted right before the instruction
    (engine streams execute in block order, so semantics are unchanged)."""
    n = 0
    for fn in nc.m.functions:
        for bb in fn.blocks:
            out = []
            for inst in bb.instructions:
                si = inst.sync_info
                if si is not None and si.on_wait and len(si.on_wait) > 1:
                    waits = list(si.on_wait)
                    si.on_wait = waits[-1:]
                    for w in waits[:-1]:
                        n += 1
                        nop = mybir.InstNoOp(
                            name=f"I-waitsplit-{n}",
                            engine=inst.engine,
                            sync_info=mybir.SyncInfo(on_wait=[w], on_update=[]),
                            text_hint="waitsplit",
                            bass_nofuse=True,
                        )
                        out.append(nop)
                out.append(inst)
            if n:
                bb.instructions = out


def _build():
    import concourse.bass as bass
    import concourse.mybir as mybir
    import concourse.tile as tile

    _patch_drain(tile, mybir)

    bf = mybir.dt.bfloat16
    f32 = mybir.dt.float32

    nc = bass.Bass()
    xa_d = nc.dram_tensor("xTa", [EC, 128, 512], bf, kind="ExternalInput")
    xb1_d = nc.dram_tensor("xTb1", [EC, 128, 512], bf, kind="ExternalInput")
    xb2_d = nc.dram_tensor("xTb2", [EC, 128, 1024], bf, kind="ExternalInput")
    # weights pre-laid host-side in partition-major order so every DMA moves
    # long contiguous lines (2-4KB) instead of 256B strided scatters
    wq_d = nc.dram_tensor("wqT", [128, 2, EC, 128], bf, kind="ExternalInput")
    wk_d = nc.dram_tensor("wkT", [128, 2, EC, 128], bf, kind="ExternalInput")
    wv_d = nc.dram_tensor("wvT", [128, EC, HDL], bf, kind="ExternalInput")
    wo_d = nc.dram_tensor("woT", [128, 2, E], bf, kind="ExternalInput")
    bq_d = nc.dram_tensor("bqs", [128, 2, 1], f32, kind="ExternalInput")
    bk_d = nc.dram_tensor("bks", [128, 2, 1], f32, kind="ExternalInput")
    y_d = [
        nc.dram_tensor("y0", [N, E], bf, kind="ExternalOutput"),
        nc.dram_tensor("y1", [N, E], bf, kind="ExternalOutput"),
    ]

    with tile.TileContext(nc) as tc:
        _emit(nc, tc, tile, mybir, xa_d, xb1_d, xb2_d, wq_d, wk_d, wv_d, wo_d, bq_d, bk_d, y_d)
    _split_excess_waits(nc, mybir)
    return nc


def _emit(nc, tc, tile, mybir, xa_d, xb1_d, xb2_d, wq_d, wk_d, wv_d, wo_d, bq_d, bk_d, y_d):
    import concourse.bass as bass
    from contextlib import ExitStack

    bf = mybir.dt.bfloat16
    f32 = mybir.dt.float32
    Exp = mybir.ActivationFunctionType.Exp

    ctx = ExitStack()
    with ctx:
        persist = ctx.enter_context(tc.tile_pool(name="persist", bufs=1))
        # PSUM budget (8 banks): en 2x2 + pv 2x1 + filler 2x1
        psen = ctx.enter_context(tc.tile_pool(name="psen", bufs=2, space="PSUM"))
        pvp = ctx.enter_context(tc.tile_pool(name="pvp", bufs=2, space="PSUM"))
        fillp = ctx.enter_context(tc.tile_pool(name="fillp", bufs=2, space="PSUM"))
        attp = ctx.enter_context(tc.tile_pool(name="attp", bufs=22))
        normp = ctx.enter_context(tc.tile_pool(name="normp", bufs=4))
        pvdp = ctx.enter_context(tc.tile_pool(name="pvdp", bufs=4))
        stagep = ctx.enter_context(tc.tile_pool(name="stagep", bufs=4))
        dramp = ctx.enter_context(tc.tile_pool(name="dramp", bufs=4, space="DRAM"))

        # ---- persistent SBUF ----
        x_sb = persist.tile([128, EC, N], bf)
        wq_sb = persist.tile([128, 2, EC, 128], bf)
        wk_sb = persist.tile([128, 2, EC, 128], bf)
        wv_sb = persist.tile([128, EC, HDL], bf)
        wo_sb = persist.tile([128, 2, E], bf)
        bq_sb = persist.tile([128, 2, 1], f32)
        bk_sb = persist.tile([128, 2, 1], f32)
        qT_sb = persist.tile([128, 2, N], bf)
        kT_sb = persist.tile([128, 2, N], bf)
        # V with per-head aug column: [V(0:64) | ones(64) | pad]
        v_sb = persist.tile([128, NT, NHL, 66], bf)
        outn_sb = persist.tile([128, 2, N], bf)
        warm_sb = persist.tile([128, 64], bf)
        warmf_in = persist.tile([1, 8], f32)
        warmf_out = persist.tile([1, 8], f32)
        ones_sb = persist.tile([1, 64], bf)

        # ---- t0: PE warm-up + exp table preload (run during input DMA) ----
        nc.vector.memset(warm_sb[:, :], 0.0)
        nc.vector.memset(warmf_in[:, :], 0.0)
        nc.vector.memset(v_sb[:, :, :, 64:65], 1.0)
        nc.vector.memset(ones_sb[:, :], 1.0)
        wacc = fillp.tile([128, 512], f32, tag="acc", name="warmacc")
        for i in range(WARM_MMS):
            nc.tensor.matmul(
                wacc[0:64, 0:64],
                lhsT=warm_sb[:, 0:64],
                rhs=warm_sb[:, 0:64],
                start=True,
                stop=True,
            )

        # ---- input DMAs, priority-ordered ----
        # weights move contiguous 2-4KB lines; the startup-critical x chunk
        # (first 512 query columns) fans out over FOUR rings so the first
        # q/k projections are fed ~2x sooner
        nc.sync.dma_start(out=wq_sb[:, 0, :, :], in_=wq_d[:, 0, :, :])
        nc.gpsimd.dma_start(out=wk_sb[:, 0, :, :], in_=wk_d[:, 0, :, :])
        nc.gpsimd.dma_start(out=bq_sb[:, :, :], in_=bq_d[:, :, :])
        nc.gpsimd.dma_start(out=bk_sb[:, :, :], in_=bk_d[:, :, :])
        xa_rings = [nc.sync, nc.gpsimd, nc.scalar]
        for ec in range(EC):
            xa_rings[ec % 3].dma_start(out=x_sb[:, ec, 0:512], in_=xa_d[ec, :, :])
        for ec in range(EC):
            e = nc.scalar if ec % 2 == 0 else nc.gpsimd
            e.dma_start(out=x_sb[:, ec, 512:1024], in_=xb1_d[ec, :, :])
        # exp table preload rides the scalar queue after its input doorbells
        nc.scalar.activation(warmf_out[:, :], warmf_in[:, :], Exp)
        nc.sync.dma_start(out=wv_sb[:, :, :], in_=wv_d[:, :, :])
        for ec in range(EC):
            e = nc.sync if ec % 2 == 0 else nc.gpsimd
            e.dma_start(out=x_sb[:, ec, 1024:2048], in_=xb2_d[ec, :, :])
        nc.gpsimd.dma_start(out=wk_sb[:, 1, :, :], in_=wk_d[:, 1, :, :])
        nc.gpsimd.dma_start(out=wq_sb[:, 1, :, :], in_=wq_d[:, 1, :, :])
        nc.sync.dma_start(out=wo_sb[:, :, :], in_=wo_d[:, :, :])

        # ---- filler group emitters (emitted in 4-MM halves so a group
        # never delays the next en by more than ~0.9us in the PE FIFO) ----
        half_state = {}

        def emit_qk_half(mat, hc, qc, part):
            dst, w_sb, b_sb = (
                (qT_sb, wq_sb, bq_sb) if mat == "q" else (kT_sb, wk_sb, bk_sb)
            )
            nsl = slice(qc * 512, (qc + 1) * 512)
            if part == 0:
                acc = fillp.tile([128, 512], f32, tag="acc", name=f"{mat}acc{hc}_{qc}")
                half_state[(mat, hc, qc)] = acc
            acc = half_state[(mat, hc, qc)]
            for ec in range(part * 4, part * 4 + 4):
                nc.tensor.matmul(
                    acc[:, :],
                    lhsT=w_sb[:, hc, ec, :],
                    rhs=x_sb[:, ec, nsl],
                    start=(ec == 0),
                    stop=(ec == EC - 1),
                )
            if part == 1:
                del half_state[(mat, hc, qc)]
                nc.vector.tensor_scalar_add(dst[:, hc, nsl], acc[:, :], b_sb[:, hc, :])

        def emit_v_half(nt, part):
            if part == 0:
                acc = fillp.tile([128, 512], f32, tag="acc", name=f"vacc{nt}")
                half_state[("v", nt)] = acc
            acc = half_state[("v", nt)]
            vacc = acc[:, 0:HDL]
            for ec in range(part * 4, part * 4 + 4):
                nc.tensor.matmul(
                    vacc,
                    lhsT=x_sb[:, ec, nt * 128 : (nt + 1) * 128],
                    rhs=wv_sb[:, ec, :],
                    start=(ec == 0),
                    stop=(ec == EC - 1),
                )
            if part == 1:
                del half_state[("v", nt)]
                nc.vector.tensor_copy(
                    out=v_sb[:, nt, :, 0:64],
                    in_=acc[:, 0:HDL].rearrange("p (h d) -> p h d", d=64),
                )

        def emit_qk_group(mat, hc, qc):
            emit_qk_half(mat, hc, qc, 0)
            emit_qk_half(mat, hc, qc, 1)

        def emit_v_group(nt):
            emit_v_half(nt, 0)
            emit_v_half(nt, 1)

        ystage_cur = {}

        def emit_out_atom(qs, hc, k, copy_eng=None, deep_psum=False):
            # one (nt, ech) micro-step of the output projection; spread one
            # per slot so the PSUM-bank recycle (via the copy) never blocks
            # the PE FIFO
            nt = qs * 4 + k // 2
            ech = k % 2
            if ech == 0:
                ystage_cur[(qs, hc)] = stagep.tile(
                    [128, E], bf, tag="ystage", name=f"ystage{nt}_{hc}"
                )
            ystage = ystage_cur[(qs, hc)]
            esl = slice(ech * 512, (ech + 1) * 512)
            if deep_psum and k % 2 == 1:
                # at the tail the en pool's banks are free: alternating pools
                # gives a 4-bank rotation so the MM->copy chain never blocks
                ent = psen.tile([128, 2, 512], f32, tag="en", name=f"oaccp{nt}_{hc}_{ech}")
                acc = ent[:, 0, :]
            else:
                acc = fillp.tile([128, 512], f32, tag="acc", name=f"oacc{nt}_{hc}_{ech}")
            nc.tensor.matmul(
                acc[:, :],
                lhsT=outn_sb[:, hc, nt * 128 : (nt + 1) * 128],
                rhs=wo_sb[:, hc, esl],
                start=True,
                stop=True,
            )
            eng = copy_eng or nc.vector
            if eng is nc.scalar:
                nc.scalar.copy(out=ystage[:, esl], in_=acc[:, :])
            else:
                eng.tensor_copy(out=ystage[:, esl], in_=acc[:, :])
            if ech == 1:
                nc.sync.dma_start(
                    out=y_d[hc][nt * 128 : (nt + 1) * 128, :], in_=ystage[:, :]
                )

        # ---- attention pipeline state ----
        att_ring = {}
        pv_tiles = {}

        def emit_en(g):
            u, kp = divmod(g, 8)
            qs, hc = UNITS[u]
            nsl = slice(qs * 512, (qs + 1) * 512)
            for h in (0, 1):
                en = psen.tile([128, 2, 512], f32, tag="en", name=f"en{g}_{h}")
                dsl = slice(h * 64, (h + 1) * 64)
                for j in (0, 1):
                    kt = 2 * kp + j
                    nc.tensor.matmul(
                        en[:, j, :],
                        lhsT=kT_sb[dsl, hc, kt * 128 : (kt + 1) * 128],
                        rhs=qT_sb[dsl, hc, nsl],
                        start=True,
                        stop=True,
                        tile_position=(h * 64, 0),
                    )
                a = attp.tile([128, 2, 512], bf, tag="att", name=f"att{g}_{h}")
                nc.scalar.activation(a[:, :, :], en[:, :, :], Exp)
                att_ring[(g, h)] = a

        def emit_pv(g):
            u, kp = divmod(g, 8)
            qs, hc = UNITS[u]
            if kp == 0:
                pv_tiles[u] = [
                    pvp.tile([128, 512], f32, tag="pv", name=f"pv{u}_{h}")
                    for h in (0, 1)
                ]
            pv = pv_tiles[u]
            for h in (0, 1):
                a = att_ring.pop((g, h))
                for j in (0, 1):
                    kt = 2 * kp + j
                    nc.tensor.matmul(
                        pv[h][0:65, :],
                        lhsT=v_sb[:, kt, hc * 2 + h, 0:65],
                        rhs=a[:, j, :],
                        start=(kp == 0 and j == 0),
                        stop=(kp == 7 and j == 1),
                    )

        pvd_tiles = {}
        rdram_tiles = {}

        def emit_norm_a(u, dq=None, chain=True):
            # drain pv PSUM -> SBUF (frees the banks for the next unit), then
            # kick off the transposed-reciprocal DMA chain
            dq = dq or nc.gpsimd
            pv = pv_tiles.pop(u)
            pvd = [pvdp.tile([128, 512], f32, tag="pvd", name=f"pvd{u}_{h}") for h in (0, 1)]
            for h in (0, 1):
                nc.vector.tensor_copy(out=pvd[h][0:65, :], in_=pv[h][0:65, :])
            pvd_tiles[u] = pvd
            if not chain:
                return
            sdram = dramp.tile([1, 1024], f32, tag="sdram")
            for h in (0, 1):
                dq.dma_start(
                    out=sdram[0:1, h * 512 : (h + 1) * 512], in_=pvd[h][64:65, :]
                )
            wide = bass.AP(
                tensor=sdram.tensor, offset=sdram.offset, ap=[[32, 32], [1, 32]]
            )
            sw = normp.tile([32, 32], f32, tag="sw")
            dq.dma_start(out=sw[0:32, :], in_=wide)
            rw = normp.tile([32, 32], f32, tag="rw")
            nc.vector.reciprocal(rw[0:32, :], sw[0:32, :])
            rdram = dramp.tile([1, 1024], f32, tag="rdram")
            wide_r = bass.AP(
                tensor=rdram.tensor, offset=rdram.offset, ap=[[32, 32], [1, 32]]
            )
            dq.dma_start(out=wide_r, in_=rw[0:32, :])
            rdram_tiles[u] = rdram

        def emit_norm_b(u, dq=None):
            qs, hc = UNITS[u]
            nsl = slice(qs * 512, (qs + 1) * 512)
            dq = dq or nc.gpsimd
            pvd = pvd_tiles.pop(u)
            rdram = rdram_tiles.pop(u)
            bcast = normp.tile([64, 1024], f32, tag="bcast")
            bsrc = bass.AP(
                tensor=rdram.tensor, offset=rdram.offset, ap=[[0, 64], [1, 1024]]
            )
            dq.dma_start(out=bcast[0:64, :], in_=bsrc)
            nc.vector.tensor_mul(
                outn_sb[0:64, hc, nsl], pvd[0][0:64, :], bcast[0:64, 0:512]
            )
            ost = normp.tile([64, 512], bf, tag="ost")
            nc.vector.tensor_mul(ost[0:64, :], pvd[1][0:64, :], bcast[0:64, 512:1024])
            nc.sync.dma_start(out=outn_sb[64:128, hc, nsl], in_=ost[0:64, :])

        def emit_norm_b_pe(u):
            # taper-unit normalization with no DRAM round trips: reciprocal of
            # the aug row on DVE, partition-broadcast via rank-1 PE matmuls
            qs, hc = UNITS[u]
            nsl = slice(qs * 512, (qs + 1) * 512)
            pvd = pvd_tiles.pop(u)
            rw = normp.tile([1, 2, 512], bf, tag="rwpe", name=f"rw{u}")
            with nc.allow_low_precision(reason="bf16 denom reciprocal, 0.4% rel"):
                for h in (0, 1):
                    nc.vector.reciprocal(rw[0:1, h, :], pvd[h][64:65, :])
            bc = [
                fillp.tile([128, 512], f32, tag="acc", name=f"bc{u}_{h}")
                for h in (0, 1)
            ]
            for h in (0, 1):
                nc.tensor.matmul(
                    bc[h][0:64, :],
                    lhsT=ones_sb[0:1, 0:64],
                    rhs=rw[0:1, h, :],
                    start=True,
                    stop=True,
                )
            nc.vector.tensor_mul(outn_sb[0:64, hc, nsl], pvd[0][0:64, :], bc[0][0:64, :])
            ost = normp.tile([64, 512], bf, tag="ost")
            nc.vector.tensor_mul(ost[0:64, :], pvd[1][0:64, :], bc[1][0:64, :])
            nc.gpsimd.dma_start(out=outn_sb[64:128, hc, nsl], in_=ost[0:64, :])

        # ---- precomputed per-slot action table ----
        # pv lag: deep (6) during U0 to shed front-loaded filler pressure,
        # 4 at steady state, tapered at the end to shrink the tail.
        def pv_slot(g):
            if g < 8:
                return g + 6
            if g < 54:
                return g + 4
            if g < 58:
                return g + 3
            return min(g + 2, 64)  # taper the lag so the tail drains fast

        PV_AT = {}
        for g in range(64):
            PV_AT.setdefault(min(pv_slot(g), 64), []).append(g)

        norm_a_slot = {}
        for u in range(8):
            norm_a_slot[u] = min(pv_slot(8 * u + 7), 64)
        norm_b_slot = {u: norm_a_slot[u] + 2 for u in range(8)}

        SLOTS = {
            1: [("k", 0, 1)],
            3: [("k", 0, 2)],
            4: [("v", 0), ("v", 1)],
            5: [("k", 0, 3), ("v", 2)],
            6: [("q", 0, 1), ("v", 3), ("v", 4)],
            7: [("v", 5), ("v", 6)],
            8: [("v", 7), ("v", 8)],
            9: [("v", 9), ("v", 10)],
            10: [("v", 11), ("q", 0, 2)],
            11: [("v", 12), ("v", 13)],
            12: [("v", 14), ("v", 15)],
            18: [("q", 0, 3)],
            24: [("q", 1, 0)],
            26: [("k", 1, 0)],
            30: [("k", 1, 1)],
            32: [("k", 1, 2)],
            34: [("k", 1, 3)],
            36: [("q", 1, 1)],
            44: [("q", 1, 2)],
            52: [("q", 1, 3)],
        }
        for u in range(8):
            for k in range(8):
                # unit 6's atoms pack 3-per-slot so they all land inside the
                # taper window, filling the PE while the last exps drain
                step = k // 3 if u == 6 else k
                s = norm_b_slot[u] + 1 + step
                if s < 64:
                    SLOTS.setdefault(s, []).append(("oa", UNITS[u][0], UNITS[u][1], k))

        # ---- preamble compute ----
        # trickle warm matmuls between the projection halves keep the PE HAM
        # from declocking across input-DMA hiccups (targets the pv pool,
        # which has no allocations until pv(0) several slots later)
        wacc_t = pvp.tile([128, 512], f32, tag="pv", name="warmtrickle")

        def warm_trickle(n):
            for _ in range(n):
                nc.tensor.matmul(
                    wacc_t[0:64, 0:64],
                    lhsT=warm_sb[:, 0:64],
                    rhs=warm_sb[:, 0:64],
                    start=True,
                    stop=True,
                )

        emit_qk_half("q", 0, 0, 0)
        warm_trickle(TRICKLE_WARM)
        emit_qk_half("k", 0, 0, 0)
        warm_trickle(TRICKLE_WARM)
        emit_qk_half("q", 0, 0, 1)
        warm_trickle(TRICKLE_WARM)
        emit_qk_half("k", 0, 0, 1)

        # ---- main pipeline ----
        emitted_atoms = set()

        def do_slot(g):
            halves = []
            for f in SLOTS.get(g, []):
                if f[0] in ("q", "k", "v"):
                    halves.append((f, 0))
                    halves.append((f, 1))
                else:
                    halves.append((f, None))

            def emit_half(item):
                f, part = item
                if f[0] in ("q", "k"):
                    emit_qk_half(*f, part)
                elif f[0] == "v":
                    emit_v_half(f[1], part)
                else:
                    emit_out_atom(*f[1:])
                    emitted_atoms.add(f[1:])

            # front-load one filler half ahead of the pv/norm section, but
            # never an out-atom: those may wait on a norm DMA and would
            # stall the in-order PE FIFO ahead of the pv matmuls
            if halves and halves[0][0][0] != "oa":
                emit_half(halves.pop(0))
            for gp in PV_AT.get(g, []):
                emit_pv(gp)
                if gp % 8 == 7 and gp // 8 in norm_a_slot:
                    # unit 7 skips the DRAM reciprocal dance; its norm runs
                    # through a PE broadcast at the tail instead
                    emit_norm_a(gp // 8, chain=(gp // 8 != 7))
                    del norm_a_slot[gp // 8]
            for u in range(7):
                if norm_b_slot.get(u) == g:
                    emit_norm_b(u)
            for item in halves:
                emit_half(item)

        for g in range(64):
            emit_en(g)
            do_slot(g)

        # ---- tail drain ----
        do_slot(64)
        # U6 atoms that didn't fit run while norm(7)'s reciprocal is in flight
        for k in range(8):
            if (2, 1, k) not in emitted_atoms:
                emit_out_atom(
                    2, 1, k, copy_eng=nc.scalar if k % 2 else nc.vector, deep_psum=True
                )
        # short HAM-warm bridge over the norm(7) reciprocal latency
        wacc2 = fillp.tile([128, 512], f32, tag="acc", name="warmacc2")
        for i in range(16):
            nc.tensor.matmul(
                wacc2[0:64, 0:256],
                lhsT=warm_sb[:, 0:64],
                rhs=x_sb[:, 0, 0:256],
                start=True,
                stop=True,
            )
        emit_norm_b7(7)
        # final unit's projection; per 128-row tile the two e-halves run as
        # FD=512 matmuls (fp32 PSUM out caps FD at one bank) with copies
        # alternating scalar/vector, and the y DMAs split across both rings
        for nt in range(12, 16):
            acc = psen.tile([128, 2, 512], f32, tag="en", name=f"oacc7_{nt}")
            ystage = stagep.tile([128, E], bf, tag="ystage", name=f"ystage7_{nt}")
            nsl_r = slice(nt * 128, (nt + 1) * 128)
            for ech in (0, 1):
                esl = slice(ech * 512, (ech + 1) * 512)
                nc.tensor.matmul(
                    acc[:, ech, :],
                    lhsT=outn_sb[:, 1, nsl_r],
                    rhs=wo_sb[:, 1, esl],
                    start=True,
                    stop=True,
                )
                if ech:
                    nc.scalar.copy(out=ystage[:, esl], in_=acc[:, ech, :])
                    nc.gpsimd.dma_start(out=y_d[1][nsl_r, esl], in_=ystage[:, esl])
                else:
                    nc.vector.tensor_copy(out=ystage[:, esl], in_=acc[:, ech, :])
                    nc.sync.dma_start(out=y_d[1][nsl_r, esl], in_=ystage[:, esl])


def _prep_core_inputs(x, Wq, bq, Wk, bk, Wv, bv, Wo, bo):
    """Build the 8 per-core input maps (host-side sharding + layout)."""
    xT_by_batch = []
    for b in range(2):
        xT = np.ascontiguousarray(x[b].T).astype(BF16).reshape(EC, 128, N)
        xa = np.ascontiguousarray(xT[:, :, 0:512])
        xb1 = np.ascontiguousarray(xT[:, :, 512:1024])
        xb2 = np.ascontiguousarray(xT[:, :, 1024:2048])
        xT_by_batch.append((xa, xb1, xb2))
    in_maps = []
    for c in range(8):
        b, g = divmod(c, 4)
        hsl = slice(g * HDL, (g + 1) * HDL)

        def pair_major(wT):
            # wT: [E, HDL] -> [128, 2, EC, 128] (partition-major, so every
            # DMA line is a contiguous 2KB run per partition)
            return np.ascontiguousarray(
                wT.reshape(EC, 128, 2, 128).transpose(1, 2, 0, 3)
            )

        wqT = pair_major((Wq[hsl, :] * SCALE).T.astype(BF16))
        wkT = pair_major(Wk[hsl, :].T.astype(BF16))
        wvT = np.ascontiguousarray(
            Wv[hsl, :].T.astype(BF16).reshape(EC, 128, HDL).transpose(1, 0, 2)
        )
        woT = np.ascontiguousarray(
            Wo[:, hsl].T.astype(BF16).reshape(2, 128, E).transpose(1, 0, 2)
        )
        bqs = np.ascontiguousarray(
            (bq[hsl] * SCALE).astype(np.float32).reshape(2, 128).T.reshape(128, 2, 1)
        )
        bks = np.ascontiguousarray(
            bk[hsl].astype(np.float32).reshape(2, 128).T.reshape(128, 2, 1)
        )
        in_maps.append(
            {
                "xTa": xT_by_batch[b][0],
                "xTb1": xT_by_batch[b][1],
                "xTb2": xT_by_batch[b][2],
                "wqT": wqT,
                "wkT": wkT,
                "wvT": wvT,
                "woT": woT,
                "bqs": bqs,
                "bks": bks,
            }
        )
    return in_maps


def run(inputs, trace=False, trace_kwargs=None):
    """Compile (cached), execute on 8 cores, gather.  Returns (y, results)."""
    from concourse.bass_utils import run_bass_kernel_spmd

    if "nc" not in _compiled:
        _compiled["nc"] = _build()
    nc = _compiled["nc"]

    in_maps = _prep_core_inputs(**inputs)
    kwargs = {}
    if trace:
        kwargs["trace"] = True
        kwargs["trace_kwargs"] = trace_kwargs or {}
    res = run_bass_kernel_spmd(nc, in_maps, core_ids=list(range(8)), **kwargs)

    x, Wo, bo, bv = inputs["x"], inputs["Wo"], inputs["bo"], inputs["bv"]
    y = np.zeros((2, N, E), np.float32)
    for c in range(8):
        b = c // 4
        y[b] += res.results[c]["y0"].astype(np.float32)
        y[b] += res.results[c]["y1"].astype(np.float32)
    y += (np.asarray(bv, np.float32) @ np.asarray(Wo, np.float32).T + np.asarray(bo, np.float32))[None, None, :]
    return y.astype(np.float32), res


def kernel(**inputs):
    inputs = {k: np.asarray(v) for k, v in inputs.items()}
    y, _ = run(inputs)
    return y



# revision 34
# speedup vs baseline: 1.0460x; 1.0347x over previous
"""Trainium2 Bass kernel for 16-head MultiHeadAttention (EMB=1024, seq=2048, batch=2).

Sharding: 8 cores = 2 batches x 4 head-groups (4 heads each).
Per core: Q/K/V projections with column-sharded weights, attention over its
4 heads, and per-head-pair partial output projections with the row-sharded
Wo.  The host sums the 8 partials per batch and adds the bv/bo terms.

Schedule: the kernel is a single software pipeline over 64 "kp" steps
(8 units of (q-chunk, head-pair) x 8 key-pair steps each).  ScalarE's exp
(~147us total) is the hard floor; every projection / output matmul is
injected as filler between attention matmuls so TensorE work (~137us)
hides completely under it.  Emission order == per-engine FIFO order.
"""

import sys

for _p in ("/opt/trn_rl_repo", "/root/.axon_site/_ro/trn_rl_repo"):
    if _p not in sys.path:
        sys.path.insert(0, _p)

import numpy as np
import ml_dtypes

BF16 = ml_dtypes.bfloat16

N = 2048          # sequence length
E = 1024          # embedding
HDL = 256         # local head width per core (4 heads x 64)
D = 64            # head dim
NHL = 4           # local heads
EC = 8            # e-chunks of 128
NT = 16           # n-tiles of 128
SCALE = 1.0 / 32.0  # 1/sqrt(E)

UNITS = [(0, 0), (1, 0), (2, 0), (3, 0), (0, 1), (1, 1), (2, 1), (3, 1)]
PV_LAG = 4
WARM_MMS = 56
TRICKLE_WARM = 6

MAX_DRAIN_WAITS = 1

_compiled = {}


def _patch_drain(tile_mod, mybir):
    """Walrus in this container rejects >1 sync wait on the final Drain;
    spread the end-of-kernel waits over nop instructions instead."""
    from concourse.vector_clock import ScopedClock

    def _drain_and_barrier(self, tick_clock, wait_clock):
        nc = self.nc
        probe = nc.sync.nop(nofuse=True)
        wait_clock.add_sem_waits(probe.ins, ScopedClock({None: tick_clock.global_clock}))
        si = probe.ins.sync_info
        waits = list(si.on_wait) if si is not None and si.on_wait else []
        if len(waits) > MAX_DRAIN_WAITS:
            si.on_wait = waits[:MAX_DRAIN_WAITS]
            rest = waits[MAX_DRAIN_WAITS:]
            for i in range(0, len(rest), MAX_DRAIN_WAITS):
                nop = nc.sync.nop(nofuse=True)
                nsi = nop.ins.sync_info
                chunk = rest[i : i + MAX_DRAIN_WAITS]
                if nsi is None:
                    nop.ins.sync_info = mybir.SyncInfo(on_wait=chunk, on_update=[])
                else:
                    nsi.on_wait = chunk
        nc.sync.drain()
        nc.all_engine_barrier()
        assert self.sems is not None
        popped = nc._tile_sem_poison_stack.pop()
        assert popped is self._sem_poison
        nc.clear_and_free_semaphores(list(self.sems.allocated().values()))
        nc.all_engine_barrier()

    tile_mod.TileContext._drain_and_barrier = _drain_and_barrier


def _split_excess_waits(nc, mybir):
    """This container's walrus rejects >1 sync wait per instruction.  Move
    extra waits onto same-engine NOPs inserted right before the instruction
    (engine streams execute in block order, so semantics are unchanged)."""
    n = 0
    for fn in nc.m.functions:
        for bb in fn.blocks:
            out = []
            for inst in bb.instructions:
                si = inst.sync_info
                if si is not None and si.on_wait and len(si.on_wait) > 1:
                    waits = list(si.on_wait)
                    si.on_wait = waits[-1:]
                    for w in waits[:-1]:
                        n += 1
                        nop = mybir.InstNoOp(
                            name=f"I-waitsplit-{n}",
                            engine=inst.engine,
                            sync_info=mybir.SyncInfo(on_wait=[w], on_update=[]),
                            text_hint="waitsplit",
                            bass_nofuse=True,
                        )
                        out.append(nop)
                out.append(inst)
            if n:
                bb.instructions = out


def _build():
    import concourse.bass as bass
    import concourse.mybir as mybir
    import concourse.tile as tile

    _patch_drain(tile, mybir)

    bf = mybir.dt.bfloat16
    f32 = mybir.dt.float32

    nc = bass.Bass()
    xa_d = nc.dram_tensor("xTa", [EC, 128, 512], bf, kind="ExternalInput")
    xb1_d = nc.dram_tensor("xTb1", [EC, 128, 512], bf, kind="ExternalInput")
    xb2_d = nc.dram_tensor("xTb2", [EC, 128, 1024], bf, kind="ExternalInput")
    # weights pre-laid host-side in partition-major order so every DMA moves
    # long contiguous lines (2-4KB) instead of 256B strided scatters
    wq_d = nc.dram_tensor("wqT", [128, 2, EC, 128], bf, kind="ExternalInput")
    wk_d = nc.dram_tensor("wkT", [128, 2, EC, 128], bf, kind="ExternalInput")
    wv_d = nc.dram_tensor("wvT", [128, EC, HDL], bf, kind="ExternalInput")
    wo_d = nc.dram_tensor("woT", [128, 2, E], bf, kind="ExternalInput")
    bq_d = nc.dram_tensor("bqs", [128, 2, 1], f32, kind="ExternalInput")
    bk_d = nc.dram_tensor("bks", [128, 2, 1], f32, kind="ExternalInput")
    y_d = [
        nc.dram_tensor("y0", [N, E], bf, kind="ExternalOutput"),
        nc.dram_tensor("y1", [N, E], bf, kind="ExternalOutput"),
    ]

    with tile.TileContext(nc) as tc:
        _emit(nc, tc, tile, mybir, xa_d, xb1_d, xb2_d, wq_d, wk_d, wv_d, wo_d, bq_d, bk_d, y_d)
    _split_excess_waits(nc, mybir)
    return nc


def _emit(nc, tc, tile, mybir, xa_d, xb1_d, xb2_d, wq_d, wk_d, wv_d, wo_d, bq_d, bk_d, y_d):
    import concourse.bass as bass
    from contextlib import ExitStack

    bf = mybir.dt.bfloat16
    f32 = mybir.dt.float32
    Exp = mybir.ActivationFunctionType.Exp

    ctx = ExitStack()
    with ctx:
        persist = ctx.enter_context(tc.tile_pool(name="persist", bufs=1))
        # PSUM budget (8 banks): en 2x2 + pv 2x1 + filler 2x1
        psen = ctx.enter_context(tc.tile_pool(name="psen", bufs=2, space="PSUM"))
        pvp = ctx.enter_context(tc.tile_pool(name="pvp", bufs=2, space="PSUM"))
        fillp = ctx.enter_context(tc.tile_pool(name="fillp", bufs=2, space="PSUM"))
        attp = ctx.enter_context(tc.tile_pool(name="attp", bufs=22))
        normp = ctx.enter_context(tc.tile_pool(name="normp", bufs=4))
        pvdp = ctx.enter_context(tc.tile_pool(name="pvdp", bufs=4))
        stagep = ctx.enter_context(tc.tile_pool(name="stagep", bufs=4))
        dramp = ctx.enter_context(tc.tile_pool(name="dramp", bufs=4, space="DRAM"))

        # ---- persistent SBUF ----
        x_sb = persist.tile([128, EC, N], bf)
        wq_sb = persist.tile([128, 2, EC, 128], bf)
        wk_sb = persist.tile([128, 2, EC, 128], bf)
        wv_sb = persist.tile([128, EC, HDL], bf)
        wo_sb = persist.tile([128, 2, E], bf)
        bq_sb = persist.tile([128, 2, 1], f32)
        bk_sb = persist.tile([128, 2, 1], f32)
        qT_sb = persist.tile([128, 2, N], bf)
        kT_sb = persist.tile([128, 2, N], bf)
        # V with per-head aug column: [V(0:64) | ones(64) | pad]
        v_sb = persist.tile([128, NT, NHL, 66], bf)
        outn_sb = persist.tile([128, 2, N], bf)
        warm_sb = persist.tile([128, 64], bf)
        warmf_in = persist.tile([1, 8], f32)
        warmf_out = persist.tile([1, 8], f32)
        ones_sb = persist.tile([1, 64], bf)

        # ---- t0: PE warm-up + exp table preload (run during input DMA) ----
        nc.vector.memset(warm_sb[:, :], 0.0)
        nc.vector.memset(warmf_in[:, :], 0.0)
        nc.vector.memset(v_sb[:, :, :, 64:65], 1.0)
        nc.vector.memset(ones_sb[:, :], 1.0)
        wacc = fillp.tile([128, 512], f32, tag="acc", name="warmacc")
        for i in range(WARM_MMS):
            nc.tensor.matmul(
                wacc[0:64, 0:64],
                lhsT=warm_sb[:, 0:64],
                rhs=warm_sb[:, 0:64],
                start=True,
                stop=True,
            )

        # ---- input DMAs, priority-ordered ----
        # weights move contiguous 2-4KB lines; the startup-critical x chunk
        # (first 512 query columns) fans out over FOUR rings so the first
        # q/k projections are fed ~2x sooner
        nc.sync.dma_start(out=wq_sb[:, 0, :, :], in_=wq_d[:, 0, :, :])
        nc.gpsimd.dma_start(out=wk_sb[:, 0, :, :], in_=wk_d[:, 0, :, :])
        nc.gpsimd.dma_start(out=bq_sb[:, :, :], in_=bq_d[:, :, :])
        nc.gpsimd.dma_start(out=bk_sb[:, :, :], in_=bk_d[:, :, :])
        xa_rings = [nc.sync, nc.gpsimd, nc.scalar]
        for ec in range(EC):
            xa_rings[ec % 3].dma_start(out=x_sb[:, ec, 0:512], in_=xa_d[ec, :, :])
        for ec in range(EC):
            e = nc.scalar if ec % 2 == 0 else nc.gpsimd
            e.dma_start(out=x_sb[:, ec, 512:1024], in_=xb1_d[ec, :, :])
        # exp table preload rides the scalar queue after its input doorbells
        nc.scalar.activation(warmf_out[:, :], warmf_in[:, :], Exp)
        nc.sync.dma_start(out=wv_sb[:, :, :], in_=wv_d[:, :, :])
        for ec in range(EC):
            e = nc.sync if ec % 2 == 0 else nc.gpsimd
            e.dma_start(out=x_sb[:, ec, 1024:2048], in_=xb2_d[ec, :, :])
        nc.gpsimd.dma_start(out=wk_sb[:, 1, :, :], in_=wk_d[:, 1, :, :])
        nc.gpsimd.dma_start(out=wq_sb[:, 1, :, :], in_=wq_d[:, 1, :, :])
        nc.sync.dma_start(out=wo_sb[:, :, :], in_=wo_d[:, :, :])

        # ---- filler group emitters (emitted in 4-MM halves so a group
        # never delays the next en by more than ~0.9us in the PE FIFO) ----
        half_state = {}

        def emit_qk_half(mat, hc, qc, part):
            dst, w_sb, b_sb = (
                (qT_sb, wq_sb, bq_sb) if mat == "q" else (kT_sb, wk_sb, bk_sb)
            )
            nsl = slice(qc * 512, (qc + 1) * 512)
            if part == 0:
                acc = fillp.tile([128, 512], f32, tag="acc", name=f"{mat}acc{hc}_{qc}")
                half_state[(mat, hc, qc)] = acc
            acc = half_state[(mat, hc, qc)]
            for ec in range(part * 4, part * 4 + 4):
                nc.tensor.matmul(
                    acc[:, :],
                    lhsT=w_sb[:, hc, ec, :],
                    rhs=x_sb[:, ec, nsl],
                    start=(ec == 0),
                    stop=(ec == EC - 1),
                )
            if part == 1:
                del half_state[(mat, hc, qc)]
                nc.vector.tensor_scalar_add(dst[:, hc, nsl], acc[:, :], b_sb[:, hc, :])

        def emit_v_half(nt, part):
            if part == 0:
                acc = fillp.tile([128, 512], f32, tag="acc", name=f"vacc{nt}")
                half_state[("v", nt)] = acc
            acc = half_state[("v", nt)]
            vacc = acc[:, 0:HDL]
            for ec in range(part * 4, part * 4 + 4):
                nc.tensor.matmul(
                    vacc,
                    lhsT=x_sb[:, ec, nt * 128 : (nt + 1) * 128],
                    rhs=wv_sb[:, ec, :],
                    start=(ec == 0),
                    stop=(ec == EC - 1),
                )
            if part == 1:
                del half_state[("v", nt)]
                nc.vector.tensor_copy(
                    out=v_sb[:, nt, :, 0:64],
                    in_=acc[:, 0:HDL].rearrange("p (h d) -> p h d", d=64),
                )

        def emit_qk_group(mat, hc, qc):
            emit_qk_half(mat, hc, qc, 0)
            emit_qk_half(mat, hc, qc, 1)

        def emit_v_group(nt):
            emit_v_half(nt, 0)
            emit_v_half(nt, 1)

        ystage_cur = {}

        def emit_out_atom(qs, hc, k, copy_eng=None, deep_psum=False):
            # one (nt, ech) micro-step of the output projection; spread one
            # per slot so the PSUM-bank recycle (via the copy) never blocks
            # the PE FIFO
            nt = qs * 4 + k // 2
            ech = k % 2
            if ech == 0:
                ystage_cur[(qs, hc)] = stagep.tile(
                    [128, E], bf, tag="ystage", name=f"ystage{nt}_{hc}"
                )
            ystage = ystage_cur[(qs, hc)]
            esl = slice(ech * 512, (ech + 1) * 512)
            if deep_psum and k % 2 == 1:
                # at the tail the en pool's banks are free: alternating pools
                # gives a 4-bank rotation so the MM->copy chain never blocks
                ent = psen.tile([128, 2, 512], f32, tag="en", name=f"oaccp{nt}_{hc}_{ech}")
                acc = ent[:, 0, :]
            else:
                acc = fillp.tile([128, 512], f32, tag="acc", name=f"oacc{nt}_{hc}_{ech}")
            nc.tensor.matmul(
                acc[:, :],
                lhsT=outn_sb[:, hc, nt * 128 : (nt + 1) * 128],
                rhs=wo_sb[:, hc, esl],
                start=True,
                stop=True,
            )
            eng = copy_eng or nc.vector
            if eng is nc.scalar:
                nc.scalar.copy(out=ystage[:, esl], in_=acc[:, :])
            else:
                eng.tensor_copy(out=ystage[:, esl], in_=acc[:, :])
            if ech == 1:
                nc.sync.dma_start(
                    out=y_d[hc][nt * 128 : (nt + 1) * 128, :], in_=ystage[:, :]
                )

        # ---- attention pipeline state ----
        att_ring = {}
        pv_tiles = {}

        def emit_en(g):
            u, kp = divmod(g, 8)
            qs, hc = UNITS[u]
            nsl = slice(qs * 512, (qs + 1) * 512)
            for h in (0, 1):
                en = psen.tile([128, 2, 512], f32, tag="en", name=f"en{g}_{h}")
                dsl = slice(h * 64, (h + 1) * 64)
                for j in (0, 1):
                    kt = 2 * kp + j
                    nc.tensor.matmul(
                        en[:, j, :],
                        lhsT=kT_sb[dsl, hc, kt * 128 : (kt + 1) * 128],
                        rhs=qT_sb[dsl, hc, nsl],
                        start=True,
                        stop=True,
                        tile_position=(h * 64, 0),
                    )
                a = attp.tile([128, 2, 512], bf, tag="att", name=f"att{g}_{h}")
                nc.scalar.activation(a[:, :, :], en[:, :, :], Exp)
                att_ring[(g, h)] = a

        def emit_pv(g):
            u, kp = divmod(g, 8)
            qs, hc = UNITS[u]
            if kp == 0:
                pv_tiles[u] = [
                    pvp.tile([128, 512], f32, tag="pv", name=f"pv{u}_{h}")
                    for h in (0, 1)
                ]
            pv = pv_tiles[u]
            for h in (0, 1):
                a = att_ring.pop((g, h))
                for j in (0, 1):
                    kt = 2 * kp + j
                    nc.tensor.matmul(
                        pv[h][0:65, :],
                        lhsT=v_sb[:, kt, hc * 2 + h, 0:65],
                        rhs=a[:, j, :],
                        start=(kp == 0 and j == 0),
                        stop=(kp == 7 and j == 1),
                    )

        pvd_tiles = {}
        rdram_tiles = {}

        def emit_norm_a(u, dq=None, chain=True):
            # drain pv PSUM -> SBUF (frees the banks for the next unit), then
            # kick off the transposed-reciprocal DMA chain
            dq = dq or nc.gpsimd
            pv = pv_tiles.pop(u)
            pvd = [pvdp.tile([128, 512], f32, tag="pvd", name=f"pvd{u}_{h}") for h in (0, 1)]
            for h in (0, 1):
                nc.vector.tensor_copy(out=pvd[h][0:65, :], in_=pv[h][0:65, :])
            pvd_tiles[u] = pvd
            if not chain:
                return
            sdram = dramp.tile([1, 1024], f32, tag="sdram")
            for h in (0, 1):
                dq.dma_start(
                    out=sdram[0:1, h * 512 : (h + 1) * 512], in_=pvd[h][64:65, :]
                )
            wide = bass.AP(
                tensor=sdram.tensor, offset=sdram.offset, ap=[[32, 32], [1, 32]]
            )
            sw = normp.tile([32, 32], f32, tag="sw")
            dq.dma_start(out=sw[0:32, :], in_=wide)
            rw = normp.tile([32, 32], f32, tag="rw")
            nc.vector.reciprocal(rw[0:32, :], sw[0:32, :])
            rdram = dramp.tile([1, 1024], f32, tag="rdram")
            wide_r = bass.AP(
                tensor=rdram.tensor, offset=rdram.offset, ap=[[32, 32], [1, 32]]
            )
            dq.dma_start(out=wide_r, in_=rw[0:32, :])
            rdram_tiles[u] = rdram

        def emit_norm_b(u, dq=None):
            qs, hc = UNITS[u]
            nsl = slice(qs * 512, (qs + 1) * 512)
            dq = dq or nc.gpsimd
            pvd = pvd_tiles.pop(u)
            rdram = rdram_tiles.pop(u)
            bcast = normp.tile([64, 1024], f32, tag="bcast")
            bsrc = bass.AP(
                tensor=rdram.tensor, offset=rdram.offset, ap=[[0, 64], [1, 1024]]
            )
            dq.dma_start(out=bcast[0:64, :], in_=bsrc)
            nc.vector.tensor_mul(
                outn_sb[0:64, hc, nsl], pvd[0][0:64, :], bcast[0:64, 0:512]
            )
            ost = normp.tile([64, 512], bf, tag="ost")
            nc.vector.tensor_mul(ost[0:64, :], pvd[1][0:64, :], bcast[0:64, 512:1024])
            nc.sync.dma_start(out=outn_sb[64:128, hc, nsl], in_=ost[0:64, :])

        def emit_norm_b_pe(u):
            # taper-unit normalization with no DRAM round trips: reciprocal of
            # the aug row on DVE, partition-broadcast via rank-1 PE matmuls
            qs, hc = UNITS[u]
            nsl = slice(qs * 512, (qs + 1) * 512)
            pvd = pvd_tiles.pop(u)
            rw = normp.tile([1, 2, 512], bf, tag="rwpe", name=f"rw{u}")
            with nc.allow_low_precision(reason="bf16 denom reciprocal, 0.4% rel"):
                for h in (0, 1):
                    nc.vector.reciprocal(rw[0:1, h, :], pvd[h][64:65, :])
            bc = [
                fillp.tile([128, 512], f32, tag="acc", name=f"bc{u}_{h}")
                for h in (0, 1)
            ]
            for h in (0, 1):
                nc.tensor.matmul(
                    bc[h][0:64, :],
                    lhsT=ones_sb[0:1, 0:64],
                    rhs=rw[0:1, h, :],
                    start=True,
                    stop=True,
                )
            nc.vector.tensor_mul(outn_sb[0:64, hc, nsl], pvd[0][0:64, :], bc[0][0:64, :])
            ost = normp.tile([64, 512], bf, tag="ost")
            nc.vector.tensor_mul(ost[0:64, :], pvd[1][0:64, :], bc[1][0:64, :])
            nc.gpsimd.dma_start(out=outn_sb[64:128, hc, nsl], in_=ost[0:64, :])

        # ---- precomputed per-slot action table ----
        # pv lag: deep (6) during U0 to shed front-loaded filler pressure,
        # 4 at steady state, tapered at the end to shrink the tail.
        def pv_slot(g):
            if g < 8:
                return g + 6
            if g < 54:
                return g + 4
            if g < 58:
                return g + 3
            return min(g + 2, 64)  # taper the lag so the tail drains fast

        PV_AT = {}
        for g in range(64):
            PV_AT.setdefault(min(pv_slot(g), 64), []).append(g)

        norm_a_slot = {}
        for u in range(8):
            norm_a_slot[u] = min(pv_slot(8 * u + 7), 64)
        norm_b_slot = {u: norm_a_slot[u] + 2 for u in range(8)}

        SLOTS = {
            1: [("k", 0, 1)],
            2: [("v", 0)],
            3: [("k", 0, 2), ("v", 1)],
            4: [("v", 2), ("v", 3)],
            5: [("k", 0, 3), ("v", 4)],
            6: [("q", 0, 1), ("v", 5)],
            7: [("v", 6), ("v", 7)],
            8: [("v", 8), ("v", 9)],
            9: [("v", 10), ("v", 11)],
            10: [("v", 12), ("q", 0, 2)],
            11: [("v", 13), ("v", 14)],
            12: [("v", 15)],
            18: [("q", 0, 3)],
            24: [("q", 1, 0)],
            26: [("k", 1, 0)],
            30: [("k", 1, 1)],
            32: [("k", 1, 2)],
            34: [("k", 1, 3)],
            36: [("q", 1, 1)],
            44: [("q", 1, 2)],
            52: [("q", 1, 3)],
        }
        for u in range(8):
            for k in range(8):
                # unit 6's atoms pack 3-per-slot so they all land inside the
                # taper window, filling the PE while the last exps drain
                step = k // 3 if u == 6 else k
                s = norm_b_slot[u] + 1 + step
                if s < 64:
                    SLOTS.setdefault(s, []).append(("oa", UNITS[u][0], UNITS[u][1], k))

        # ---- preamble compute ----
        # trickle warm matmuls between the projection halves keep the PE HAM
        # from declocking across input-DMA hiccups (targets the pv pool,
        # which has no allocations until pv(0) several slots later)
        wacc_t = pvp.tile([128, 512], f32, tag="pv", name="warmtrickle")

        def warm_trickle(n):
            for _ in range(n):
                nc.tensor.matmul(
                    wacc_t[0:64, 0:64],
                    lhsT=warm_sb[:, 0:64],
                    rhs=warm_sb[:, 0:64],
                    start=True,
                    stop=True,
                )

        emit_qk_half("q", 0, 0, 0)
        warm_trickle(TRICKLE_WARM)
        emit_qk_half("k", 0, 0, 0)
        warm_trickle(TRICKLE_WARM)
        emit_qk_half("q", 0, 0, 1)
        warm_trickle(TRICKLE_WARM)
        emit_qk_half("k", 0, 0, 1)

        # ---- main pipeline ----
        emitted_atoms = set()

        def do_slot(g):
            halves = []
            for f in SLOTS.get(g, []):
                if f[0] in ("q", "k", "v"):
                    halves.append((f, 0))
                    halves.append((f, 1))
                else:
                    halves.append((f, None))

            def emit_half(item):
                f, part = item
                if f[0] in ("q", "k"):
                    emit_qk_half(*f, part)
                elif f[0] == "v":
                    emit_v_half(f[1], part)
                else:
                    emit_out_atom(*f[1:])
                    emitted_atoms.add(f[1:])

            # front-load one filler half ahead of the pv/norm section, but
            # never an out-atom: those may wait on a norm DMA and would
            # stall the in-order PE FIFO ahead of the pv matmuls
            if halves and halves[0][0][0] != "oa":
                emit_half(halves.pop(0))
            for gp in PV_AT.get(g, []):
                emit_pv(gp)
                if gp % 8 == 7 and gp // 8 in norm_a_slot:
                    # units 6 and 7 skip the DRAM reciprocal dance; their
                    # norms run through PE broadcasts (emit_norm_b_pe)
                    emit_norm_a(gp // 8, chain=(gp // 8 < 6))
                    del norm_a_slot[gp // 8]
            for u in range(7):
                if norm_b_slot.get(u) == g:
                    if u == 6:
                        emit_norm_b_pe(u)
                    else:
                        emit_norm_b(u)
            for item in halves:
                emit_half(item)
            # a few warm matmuls in the first slots bridge the early
            # pipeline-fill stalls so the HAM never declocks the body
            if 1 <= g <= 4:
                warm_trickle(4)

        for g in range(64):
            emit_en(g)
            do_slot(g)

        # ---- tail drain ----
        do_slot(64)
        # U6 atoms that didn't fit run while norm(7)'s reciprocal is in flight
        for k in range(8):
            if (2, 1, k) not in emitted_atoms:
                emit_out_atom(
                    2, 1, k, copy_eng=nc.scalar if k % 2 else nc.vector, deep_psum=True
                )
        # short HAM-warm bridge over the norm(7) reciprocal latency
        wacc2 = fillp.tile([128, 512], f32, tag="acc", name="warmacc2")
        for i in range(16):
            nc.tensor.matmul(
                wacc2[0:64, 0:256],
                lhsT=warm_sb[:, 0:64],
                rhs=x_sb[:, 0, 0:256],
                start=True,
                stop=True,
            )
        emit_norm_b_pe(7)
        # final unit's projection; per 128-row tile the two e-halves run as
        # FD=512 matmuls (fp32 PSUM out caps FD at one bank) with copies
        # alternating scalar/vector, and the y DMAs split across both rings;
        # accs alternate psen/fillp so they never wait on the same recycle
        for nt in range(12, 16):
            if nt % 2 == 0:
                ent = psen.tile([128, 2, 512], f32, tag="en", name=f"oacc7_{nt}")
                accs = [ent[:, 0, :], ent[:, 1, :]]
            else:
                accs = [
                    fillp.tile([128, 512], f32, tag="acc", name=f"oacc7_{nt}_{e}")
                    for e in (0, 1)
                ]
            ystage = stagep.tile([128, E], bf, tag="ystage", name=f"ystage7_{nt}")
            nsl_r = slice(nt * 128, (nt + 1) * 128)
            for ech in (0, 1):
                esl = slice(ech * 512, (ech + 1) * 512)
                nc.tensor.matmul(
                    accs[ech],
                    lhsT=outn_sb[:, 1, nsl_r],
                    rhs=wo_sb[:, 1, esl],
                    start=True,
                    stop=True,
                )
                if ech:
                    nc.scalar.copy(out=ystage[:, esl], in_=accs[ech])
                    nc.gpsimd.dma_start(out=y_d[1][nsl_r, esl], in_=ystage[:, esl])
                else:
                    nc.vector.tensor_copy(out=ystage[:, esl], in_=accs[ech])
                    nc.sync.dma_start(out=y_d[1][nsl_r, esl], in_=ystage[:, esl])


def _prep_core_inputs(x, Wq, bq, Wk, bk, Wv, bv, Wo, bo):
    """Build the 8 per-core input maps (host-side sharding + layout)."""
    xT_by_batch = []
    for b in range(2):
        xT = np.ascontiguousarray(x[b].T).astype(BF16).reshape(EC, 128, N)
        xa = np.ascontiguousarray(xT[:, :, 0:512])
        xb1 = np.ascontiguousarray(xT[:, :, 512:1024])
        xb2 = np.ascontiguousarray(xT[:, :, 1024:2048])
        xT_by_batch.append((xa, xb1, xb2))
    in_maps = []
    for c in range(8):
        b, g = divmod(c, 4)
        hsl = slice(g * HDL, (g + 1) * HDL)

        def pair_major(wT):
            # wT: [E, HDL] -> [128, 2, EC, 128] (partition-major, so every
            # DMA line is a contiguous 2KB run per partition)
            return np.ascontiguousarray(
                wT.reshape(EC, 128, 2, 128).transpose(1, 2, 0, 3)
            )

        wqT = pair_major((Wq[hsl, :] * SCALE).T.astype(BF16))
        wkT = pair_major(Wk[hsl, :].T.astype(BF16))
        wvT = np.ascontiguousarray(
            Wv[hsl, :].T.astype(BF16).reshape(EC, 128, HDL).transpose(1, 0, 2)
        )
        woT = np.ascontiguousarray(
            Wo[:, hsl].T.astype(BF16).reshape(2, 128, E).transpose(1, 0, 2)
        )
        bqs = np.ascontiguousarray(
            (bq[hsl] * SCALE).astype(np.float32).reshape(2, 128).T.reshape(128, 2, 1)
        )
        bks = np.ascontiguousarray(
            bk[hsl].astype(np.float32).reshape(2, 128).T.reshape(128, 2, 1)
        )
        in_maps.append(
            {
                "xTa": xT_by_batch[b][0],
                "xTb1": xT_by_batch[b][1],
                "xTb2": xT_by_batch[b][2],
                "wqT": wqT,
                "wkT": wkT,
                "wvT": wvT,
                "woT": woT,
                "bqs": bqs,
                "bks": bks,
            }
        )
    return in_maps


def run(inputs, trace=False, trace_kwargs=None):
    """Compile (cached), execute on 8 cores, gather.  Returns (y, results)."""
    from concourse.bass_utils import run_bass_kernel_spmd

    if "nc" not in _compiled:
        _compiled["nc"] = _build()
    nc = _compiled["nc"]

    in_maps = _prep_core_inputs(**inputs)
    kwargs = {}
    if trace:
        kwargs["trace"] = True
        kwargs["trace_kwargs"] = trace_kwargs or {}
    res = run_bass_kernel_spmd(nc, in_maps, core_ids=list(range(8)), **kwargs)

    x, Wo, bo, bv = inputs["x"], inputs["Wo"], inputs["bo"], inputs["bv"]
    y = np.zeros((2, N, E), np.float32)
    for c in range(8):
        b = c // 4
        y[b] += res.results[c]["y0"].astype(np.float32)
        y[b] += res.results[c]["y1"].astype(np.float32)
    y += (np.asarray(bv, np.float32) @ np.asarray(Wo, np.float32).T + np.asarray(bo, np.float32))[None, None, :]
    return y.astype(np.float32), res


def kernel(**inputs):
    inputs = {k: np.asarray(v) for k, v in inputs.items()}
    y, _ = run(inputs)
    return y



# revision 38
# speedup vs baseline: 1.0847x; 1.0370x over previous
"""Trainium2 Bass kernel for 16-head MultiHeadAttention (EMB=1024, seq=2048, batch=2).

Sharding: 8 cores = 2 batches x 4 head-groups (4 heads each).
Per core: Q/K/V projections with column-sharded weights, attention over its
4 heads, and per-head-pair partial output projections with the row-sharded
Wo.  The host sums the 8 partials per batch and adds the bv/bo terms.

Schedule: the kernel is a single software pipeline over 64 "kp" steps
(8 units of (q-chunk, head-pair) x 8 key-pair steps each).  ScalarE's exp
(~147us total) is the hard floor; every projection / output matmul is
injected as filler between attention matmuls so TensorE work (~137us)
hides completely under it.  Emission order == per-engine FIFO order.
"""

import sys

for _p in ("/opt/trn_rl_repo", "/root/.axon_site/_ro/trn_rl_repo"):
    if _p not in sys.path:
        sys.path.insert(0, _p)

import numpy as np
import ml_dtypes

BF16 = ml_dtypes.bfloat16

N = 2048          # sequence length
E = 1024          # embedding
HDL = 256         # local head width per core (4 heads x 64)
D = 64            # head dim
NHL = 4           # local heads
EC = 8            # e-chunks of 128
NT = 16           # n-tiles of 128
SCALE = 1.0 / 32.0  # 1/sqrt(E)

UNITS = [(0, 0), (1, 0), (2, 0), (3, 0), (0, 1), (1, 1), (2, 1), (3, 1)]
PV_LAG = 4
WARM_MMS = 56
TRICKLE_WARM = 6

MAX_DRAIN_WAITS = 1

_compiled = {}


def _patch_drain(tile_mod, mybir):
    """Walrus in this container rejects >1 sync wait on the final Drain;
    spread the end-of-kernel waits over nop instructions instead."""
    from concourse.vector_clock import ScopedClock

    def _drain_and_barrier(self, tick_clock, wait_clock):
        nc = self.nc
        probe = nc.sync.nop(nofuse=True)
        wait_clock.add_sem_waits(probe.ins, ScopedClock({None: tick_clock.global_clock}))
        si = probe.ins.sync_info
        waits = list(si.on_wait) if si is not None and si.on_wait else []
        if len(waits) > MAX_DRAIN_WAITS:
            si.on_wait = waits[:MAX_DRAIN_WAITS]
            rest = waits[MAX_DRAIN_WAITS:]
            for i in range(0, len(rest), MAX_DRAIN_WAITS):
                nop = nc.sync.nop(nofuse=True)
                nsi = nop.ins.sync_info
                chunk = rest[i : i + MAX_DRAIN_WAITS]
                if nsi is None:
                    nop.ins.sync_info = mybir.SyncInfo(on_wait=chunk, on_update=[])
                else:
                    nsi.on_wait = chunk
        nc.sync.drain()
        nc.all_engine_barrier()
        assert self.sems is not None
        popped = nc._tile_sem_poison_stack.pop()
        assert popped is self._sem_poison
        nc.clear_and_free_semaphores(list(self.sems.allocated().values()))
        nc.all_engine_barrier()

    tile_mod.TileContext._drain_and_barrier = _drain_and_barrier


def _split_excess_waits(nc, mybir):
    """This container's walrus rejects >1 sync wait per instruction.  Move
    extra waits onto same-engine NOPs inserted right before the instruction
    (engine streams execute in block order, so semantics are unchanged)."""
    n = 0
    for fn in nc.m.functions:
        for bb in fn.blocks:
            out = []
            for inst in bb.instructions:
                si = inst.sync_info
                if si is not None and si.on_wait and len(si.on_wait) > 1:
                    waits = list(si.on_wait)
                    si.on_wait = waits[-1:]
                    for w in waits[:-1]:
                        n += 1
                        nop = mybir.InstNoOp(
                            name=f"I-waitsplit-{n}",
                            engine=inst.engine,
                            sync_info=mybir.SyncInfo(on_wait=[w], on_update=[]),
                            text_hint="waitsplit",
                            bass_nofuse=True,
                        )
                        out.append(nop)
                out.append(inst)
            if n:
                bb.instructions = out


def _build():
    import concourse.bass as bass
    import concourse.mybir as mybir
    import concourse.tile as tile

    _patch_drain(tile, mybir)

    bf = mybir.dt.bfloat16
    f32 = mybir.dt.float32

    nc = bass.Bass()
    xa_d = nc.dram_tensor("xTa", [EC, 128, 512], bf, kind="ExternalInput")
    xb1_d = nc.dram_tensor("xTb1", [EC, 128, 512], bf, kind="ExternalInput")
    xb2_d = nc.dram_tensor("xTb2", [EC, 128, 1024], bf, kind="ExternalInput")
    # weights pre-laid host-side in partition-major order so every DMA moves
    # long contiguous lines (2-4KB) instead of 256B strided scatters
    wq_d = nc.dram_tensor("wqT", [128, 2, EC, 128], bf, kind="ExternalInput")
    wk_d = nc.dram_tensor("wkT", [128, 2, EC, 128], bf, kind="ExternalInput")
    wv_d = nc.dram_tensor("wvT", [128, EC, HDL], bf, kind="ExternalInput")
    wo_d = nc.dram_tensor("woT", [128, 2, E], bf, kind="ExternalInput")
    bq_d = nc.dram_tensor("bqs", [128, 2, 1], f32, kind="ExternalInput")
    bk_d = nc.dram_tensor("bks", [128, 2, 1], f32, kind="ExternalInput")
    y_d = [
        nc.dram_tensor("y0", [N, E], bf, kind="ExternalOutput"),
        nc.dram_tensor("y1", [N, E], bf, kind="ExternalOutput"),
    ]

    with tile.TileContext(nc) as tc:
        _emit(nc, tc, tile, mybir, xa_d, xb1_d, xb2_d, wq_d, wk_d, wv_d, wo_d, bq_d, bk_d, y_d)
    _split_excess_waits(nc, mybir)
    return nc


def _emit(nc, tc, tile, mybir, xa_d, xb1_d, xb2_d, wq_d, wk_d, wv_d, wo_d, bq_d, bk_d, y_d):
    import concourse.bass as bass
    from contextlib import ExitStack

    bf = mybir.dt.bfloat16
    f32 = mybir.dt.float32
    Exp = mybir.ActivationFunctionType.Exp

    ctx = ExitStack()
    with ctx:
        persist = ctx.enter_context(tc.tile_pool(name="persist", bufs=1))
        # PSUM budget (8 banks): en 2x2 + pv 2x1 + filler 2x1
        psen = ctx.enter_context(tc.tile_pool(name="psen", bufs=2, space="PSUM"))
        pvp = ctx.enter_context(tc.tile_pool(name="pvp", bufs=2, space="PSUM"))
        fillp = ctx.enter_context(tc.tile_pool(name="fillp", bufs=2, space="PSUM"))
        attp = ctx.enter_context(tc.tile_pool(name="attp", bufs=22))
        normp = ctx.enter_context(tc.tile_pool(name="normp", bufs=4))
        pvdp = ctx.enter_context(tc.tile_pool(name="pvdp", bufs=4))
        stagep = ctx.enter_context(tc.tile_pool(name="stagep", bufs=4))
        dramp = ctx.enter_context(tc.tile_pool(name="dramp", bufs=4, space="DRAM"))

        # ---- persistent SBUF ----
        x_sb = persist.tile([128, EC, N], bf)
        wq_sb = persist.tile([128, 2, EC, 128], bf)
        wk_sb = persist.tile([128, 2, EC, 128], bf)
        wv_sb = persist.tile([128, EC, HDL], bf)
        wo_sb = persist.tile([128, 2, E], bf)
        bq_sb = persist.tile([128, 2, 1], f32)
        bk_sb = persist.tile([128, 2, 1], f32)
        qT_sb = persist.tile([128, 2, N], bf)
        kT_sb = persist.tile([128, 2, N], bf)
        # V with per-head aug column: [V(0:64) | ones(64) | pad]
        v_sb = persist.tile([128, NT, NHL, 66], bf)
        outn_sb = persist.tile([128, 2, N], bf)
        warm_sb = persist.tile([128, 64], bf)
        warmf_in = persist.tile([1, 8], f32)
        warmf_out = persist.tile([1, 8], f32)
        ones_sb = persist.tile([1, 64], bf)

        # ---- t0: PE warm-up + exp table preload (run during input DMA) ----
        nc.vector.memset(warm_sb[:, :], 0.0)
        nc.vector.memset(warmf_in[:, :], 0.0)
        nc.vector.memset(v_sb[:, :, :, 64:65], 1.0)
        nc.vector.memset(ones_sb[:, :], 1.0)
        wacc = fillp.tile([128, 512], f32, tag="acc", name="warmacc")
        for i in range(WARM_MMS):
            nc.tensor.matmul(
                wacc[0:64, 0:64],
                lhsT=warm_sb[:, 0:64],
                rhs=warm_sb[:, 0:64],
                start=True,
                stop=True,
            )

        # ---- input DMAs, priority-ordered ----
        # weights move contiguous 2-4KB lines; the startup-critical x chunk
        # (first 512 query columns) fans out over FOUR rings so the first
        # q/k projections are fed ~2x sooner
        nc.sync.dma_start(out=wq_sb[:, 0, :, :], in_=wq_d[:, 0, :, :])
        nc.gpsimd.dma_start(out=wk_sb[:, 0, :, :], in_=wk_d[:, 0, :, :])
        nc.gpsimd.dma_start(out=bq_sb[:, :, :], in_=bq_d[:, :, :])
        nc.gpsimd.dma_start(out=bk_sb[:, :, :], in_=bk_d[:, :, :])
        xa_rings = [nc.sync, nc.gpsimd, nc.scalar]
        for ec in range(EC):
            xa_rings[ec % 3].dma_start(out=x_sb[:, ec, 0:512], in_=xa_d[ec, :, :])
        for ec in range(EC):
            e = nc.scalar if ec % 2 == 0 else nc.gpsimd
            e.dma_start(out=x_sb[:, ec, 512:1024], in_=xb1_d[ec, :, :])
        # exp table preload rides the scalar queue after its input doorbells
        nc.scalar.activation(warmf_out[:, :], warmf_in[:, :], Exp)
        nc.sync.dma_start(out=wv_sb[:, :, :], in_=wv_d[:, :, :])
        for ec in range(EC):
            e = nc.sync if ec % 2 == 0 else nc.gpsimd
            e.dma_start(out=x_sb[:, ec, 1024:2048], in_=xb2_d[ec, :, :])
        nc.gpsimd.dma_start(out=wk_sb[:, 1, :, :], in_=wk_d[:, 1, :, :])
        nc.gpsimd.dma_start(out=wq_sb[:, 1, :, :], in_=wq_d[:, 1, :, :])
        nc.sync.dma_start(out=wo_sb[:, :, :], in_=wo_d[:, :, :])

        # ---- filler group emitters (emitted in 4-MM halves so a group
        # never delays the next en by more than ~0.9us in the PE FIFO) ----
        half_state = {}

        def emit_qk_half(mat, hc, qc, part):
            dst, w_sb, b_sb = (
                (qT_sb, wq_sb, bq_sb) if mat == "q" else (kT_sb, wk_sb, bk_sb)
            )
            nsl = slice(qc * 512, (qc + 1) * 512)
            if part == 0:
                acc = fillp.tile([128, 512], f32, tag="acc", name=f"{mat}acc{hc}_{qc}")
                half_state[(mat, hc, qc)] = acc
            acc = half_state[(mat, hc, qc)]
            for ec in range(part * 4, part * 4 + 4):
                nc.tensor.matmul(
                    acc[:, :],
                    lhsT=w_sb[:, hc, ec, :],
                    rhs=x_sb[:, ec, nsl],
                    start=(ec == 0),
                    stop=(ec == EC - 1),
                )
            if part == 1:
                del half_state[(mat, hc, qc)]
                nc.vector.tensor_scalar_add(dst[:, hc, nsl], acc[:, :], b_sb[:, hc, :])

        def emit_v_half(nt, part):
            if part == 0:
                acc = fillp.tile([128, 512], f32, tag="acc", name=f"vacc{nt}")
                half_state[("v", nt)] = acc
            acc = half_state[("v", nt)]
            vacc = acc[:, 0:HDL]
            for ec in range(part * 4, part * 4 + 4):
                nc.tensor.matmul(
                    vacc,
                    lhsT=x_sb[:, ec, nt * 128 : (nt + 1) * 128],
                    rhs=wv_sb[:, ec, :],
                    start=(ec == 0),
                    stop=(ec == EC - 1),
                )
            if part == 1:
                del half_state[("v", nt)]
                nc.vector.tensor_copy(
                    out=v_sb[:, nt, :, 0:64],
                    in_=acc[:, 0:HDL].rearrange("p (h d) -> p h d", d=64),
                )

        def emit_qk_group(mat, hc, qc):
            emit_qk_half(mat, hc, qc, 0)
            emit_qk_half(mat, hc, qc, 1)

        def emit_v_group(nt):
            emit_v_half(nt, 0)
            emit_v_half(nt, 1)

        ystage_cur = {}

        def emit_out_atom(qs, hc, k, copy_eng=None, deep_psum=False):
            # one (nt, ech) micro-step of the output projection; spread one
            # per slot so the PSUM-bank recycle (via the copy) never blocks
            # the PE FIFO
            nt = qs * 4 + k // 2
            ech = k % 2
            if ech == 0:
                ystage_cur[(qs, hc)] = stagep.tile(
                    [128, E], bf, tag="ystage", name=f"ystage{nt}_{hc}"
                )
            ystage = ystage_cur[(qs, hc)]
            esl = slice(ech * 512, (ech + 1) * 512)
            if deep_psum and k % 2 == 1:
                # at the tail the en pool's banks are free: alternating pools
                # gives a 4-bank rotation so the MM->copy chain never blocks
                ent = psen.tile([128, 2, 512], f32, tag="en", name=f"oaccp{nt}_{hc}_{ech}")
                acc = ent[:, 0, :]
            else:
                acc = fillp.tile([128, 512], f32, tag="acc", name=f"oacc{nt}_{hc}_{ech}")
            nc.tensor.matmul(
                acc[:, :],
                lhsT=outn_sb[:, hc, nt * 128 : (nt + 1) * 128],
                rhs=wo_sb[:, hc, esl],
                start=True,
                stop=True,
            )
            eng = copy_eng or nc.vector
            if eng is nc.scalar:
                nc.scalar.copy(out=ystage[:, esl], in_=acc[:, :])
            else:
                eng.tensor_copy(out=ystage[:, esl], in_=acc[:, :])
            if ech == 1:
                nc.sync.dma_start(
                    out=y_d[hc][nt * 128 : (nt + 1) * 128, :], in_=ystage[:, :]
                )

        # ---- attention pipeline state ----
        att_ring = {}
        pv_tiles = {}

        def emit_en(g):
            # one group per key tile; both heads share the 2-bank PSUM tile
            # (subtile per head), so with bufs=2 consecutive groups are truly
            # double-buffered: en(g+2) waits on exp(g), not exp(g+1)
            u, kt = divmod(g, 16)
            qs, hc = UNITS[u]
            nsl = slice(qs * 512, (qs + 1) * 512)
            en = psen.tile([128, 2, 512], f32, tag="en", name=f"en{g}")
            for h in (0, 1):
                dsl = slice(h * 64, (h + 1) * 64)
                nc.tensor.matmul(
                    en[:, h, :],
                    lhsT=kT_sb[dsl, hc, kt * 128 : (kt + 1) * 128],
                    rhs=qT_sb[dsl, hc, nsl],
                    start=True,
                    stop=True,
                    tile_position=(h * 64, 0),
                )
            a = attp.tile([128, 2, 512], bf, tag="att", name=f"att{g}")
            nc.scalar.activation(a[:, :, :], en[:, :, :], Exp)
            att_ring[g] = a

        def emit_pv(g):
            u, kt = divmod(g, 16)
            qs, hc = UNITS[u]
            if kt == 0:
                pv_tiles[u] = [
                    pvp.tile([128, 512], f32, tag="pv", name=f"pv{u}_{h}")
                    for h in (0, 1)
                ]
            pv = pv_tiles[u]
            a = att_ring.pop(g)
            for h in (0, 1):
                nc.tensor.matmul(
                    pv[h][0:65, :],
                    lhsT=v_sb[:, kt, hc * 2 + h, 0:65],
                    rhs=a[:, h, :],
                    start=(kt == 0),
                    stop=(kt == 15),
                )

        pvd_tiles = {}
        rdram_tiles = {}

        def emit_norm_a(u, dq=None, chain=True):
            # drain pv PSUM -> SBUF (frees the banks for the next unit), then
            # kick off the transposed-reciprocal DMA chain
            dq = dq or nc.gpsimd
            pv = pv_tiles.pop(u)
            pvd = [pvdp.tile([128, 512], f32, tag="pvd", name=f"pvd{u}_{h}") for h in (0, 1)]
            for h in (0, 1):
                nc.vector.tensor_copy(out=pvd[h][0:65, :], in_=pv[h][0:65, :])
            pvd_tiles[u] = pvd
            if not chain:
                return
            sdram = dramp.tile([1, 1024], f32, tag="sdram")
            for h in (0, 1):
                dq.dma_start(
                    out=sdram[0:1, h * 512 : (h + 1) * 512], in_=pvd[h][64:65, :]
                )
            wide = bass.AP(
                tensor=sdram.tensor, offset=sdram.offset, ap=[[32, 32], [1, 32]]
            )
            sw = normp.tile([32, 32], f32, tag="sw")
            dq.dma_start(out=sw[0:32, :], in_=wide)
            rw = normp.tile([32, 32], f32, tag="rw")
            nc.vector.reciprocal(rw[0:32, :], sw[0:32, :])
            rdram = dramp.tile([1, 1024], f32, tag="rdram")
            wide_r = bass.AP(
                tensor=rdram.tensor, offset=rdram.offset, ap=[[32, 32], [1, 32]]
            )
            dq.dma_start(out=wide_r, in_=rw[0:32, :])
            rdram_tiles[u] = rdram

        def emit_norm_b(u, dq=None):
            qs, hc = UNITS[u]
            nsl = slice(qs * 512, (qs + 1) * 512)
            dq = dq or nc.gpsimd
            pvd = pvd_tiles.pop(u)
            rdram = rdram_tiles.pop(u)
            bcast = normp.tile([64, 1024], f32, tag="bcast")
            bsrc = bass.AP(
                tensor=rdram.tensor, offset=rdram.offset, ap=[[0, 64], [1, 1024]]
            )
            dq.dma_start(out=bcast[0:64, :], in_=bsrc)
            nc.vector.tensor_mul(
                outn_sb[0:64, hc, nsl], pvd[0][0:64, :], bcast[0:64, 0:512]
            )
            ost = normp.tile([64, 512], bf, tag="ost")
            nc.vector.tensor_mul(ost[0:64, :], pvd[1][0:64, :], bcast[0:64, 512:1024])
            nc.sync.dma_start(out=outn_sb[64:128, hc, nsl], in_=ost[0:64, :])

        def emit_norm_b_pe(u):
            # taper-unit normalization with no DRAM round trips: reciprocal of
            # the aug row on DVE, partition-broadcast via rank-1 PE matmuls
            qs, hc = UNITS[u]
            nsl = slice(qs * 512, (qs + 1) * 512)
            pvd = pvd_tiles.pop(u)
            rw = normp.tile([1, 2, 512], bf, tag="rwpe", name=f"rw{u}")
            with nc.allow_low_precision(reason="bf16 denom reciprocal, 0.4% rel"):
                for h in (0, 1):
                    nc.vector.reciprocal(rw[0:1, h, :], pvd[h][64:65, :])
            bc = [
                fillp.tile([128, 512], f32, tag="acc", name=f"bc{u}_{h}")
                for h in (0, 1)
            ]
            for h in (0, 1):
                nc.tensor.matmul(
                    bc[h][0:64, :],
                    lhsT=ones_sb[0:1, 0:64],
                    rhs=rw[0:1, h, :],
                    start=True,
                    stop=True,
                )
            nc.vector.tensor_mul(outn_sb[0:64, hc, nsl], pvd[0][0:64, :], bc[0][0:64, :])
            ost = normp.tile([64, 512], bf, tag="ost")
            nc.vector.tensor_mul(ost[0:64, :], pvd[1][0:64, :], bc[1][0:64, :])
            nc.gpsimd.dma_start(out=outn_sb[64:128, hc, nsl], in_=ost[0:64, :])

        # ---- precomputed per-slot action table (slots are per key tile) ----
        # pv lag: deep (12) during U0 to shed front-loaded filler pressure,
        # 8 at steady state, tapered at the end so the drain overlaps; the
        # taper keeps slots monotone so units' pv accumulations stay ordered
        def pv_slot(g):
            if g < 16:
                return g + 12
            if g < 104:
                return g + 8
            if g < 112:
                return g + 7
            if g < 120:
                return g + 6
            return min(g + 5, 128)

        PV_AT = {}
        for g in range(128):
            PV_AT.setdefault(min(pv_slot(g), 128), []).append(g)

        norm_a_slot = {}
        for u in range(8):
            norm_a_slot[u] = min(pv_slot(16 * u + 15), 128)
        norm_b_slot = {u: norm_a_slot[u] + 4 for u in range(8)}

        SLOTS = {
            2: [("k", 0, 1)],
            4: [("v", 0)],
            6: [("k", 0, 2), ("v", 1)],
            8: [("v", 2), ("v", 3)],
            10: [("k", 0, 3), ("v", 4)],
            12: [("q", 0, 1), ("v", 5)],
            14: [("v", 6), ("v", 7)],
            16: [("v", 8), ("v", 9)],
            18: [("v", 10), ("v", 11)],
            20: [("v", 12), ("q", 0, 2)],
            22: [("v", 13), ("v", 14)],
            24: [("v", 15)],
            36: [("q", 0, 3)],
            48: [("q", 1, 0)],
            52: [("k", 1, 0)],
            60: [("k", 1, 1)],
            64: [("k", 1, 2)],
            68: [("k", 1, 3)],
            72: [("q", 1, 1)],
            88: [("q", 1, 2)],
            104: [("q", 1, 3)],
        }
        for u in range(8):
            for k in range(8):
                # unit 6's atoms pack 3-per-slot-pair so they all land inside
                # the taper window, filling the PE while the last exps drain
                step = 2 * (k // 3) if u == 6 else 2 * k
                s = norm_b_slot[u] + 1 + step
                if s < 128:
                    SLOTS.setdefault(s, []).append(("oa", UNITS[u][0], UNITS[u][1], k))

        # ---- preamble compute ----
        # trickle warm matmuls between the projection halves keep the PE HAM
        # from declocking across input-DMA hiccups (targets the pv pool,
        # which has no allocations until pv(0) several slots later)
        wacc_t = pvp.tile([128, 512], f32, tag="pv", name="warmtrickle")

        def warm_trickle(n):
            for _ in range(n):
                nc.tensor.matmul(
                    wacc_t[0:64, 0:64],
                    lhsT=warm_sb[:, 0:64],
                    rhs=warm_sb[:, 0:64],
                    start=True,
                    stop=True,
                )

        emit_qk_half("q", 0, 0, 0)
        warm_trickle(TRICKLE_WARM)
        emit_qk_half("k", 0, 0, 0)
        warm_trickle(TRICKLE_WARM)
        emit_qk_half("q", 0, 0, 1)
        warm_trickle(TRICKLE_WARM)
        emit_qk_half("k", 0, 0, 1)

        # ---- main pipeline ----
        emitted_atoms = set()

        def do_slot(g):
            halves = []
            for f in SLOTS.get(g, []):
                if f[0] in ("q", "k", "v"):
                    halves.append((f, 0))
                    halves.append((f, 1))
                else:
                    halves.append((f, None))

            def emit_half(item):
                f, part = item
                if f[0] in ("q", "k"):
                    emit_qk_half(*f, part)
                elif f[0] == "v":
                    emit_v_half(f[1], part)
                else:
                    emit_out_atom(*f[1:])
                    emitted_atoms.add(f[1:])

            # front-load one filler half ahead of the pv/norm section, but
            # never an out-atom: those may wait on a norm DMA and would
            # stall the in-order PE FIFO ahead of the pv matmuls
            if halves and halves[0][0][0] != "oa":
                emit_half(halves.pop(0))
            for gp in PV_AT.get(g, []):
                emit_pv(gp)
                if gp % 16 == 15 and gp // 16 in norm_a_slot:
                    # units 6 and 7 skip the DRAM reciprocal dance; their
                    # norms run through PE broadcasts (emit_norm_b_pe)
                    emit_norm_a(gp // 16, chain=(gp // 16 < 6))
                    del norm_a_slot[gp // 16]
            for u in range(7):
                if norm_b_slot.get(u) == g:
                    if u == 6:
                        emit_norm_b_pe(u)
                    else:
                        emit_norm_b(u)
            for item in halves:
                emit_half(item)
            # a few warm matmuls in the first slots bridge the early
            # pipeline-fill stalls so the HAM never declocks the body
            if 1 <= g <= 8:
                warm_trickle(2)

        for g in range(128):
            emit_en(g)
            do_slot(g)

        # ---- tail drain ----
        do_slot(128)
        # U6 atoms that didn't fit run while norm(7)'s reciprocal is in flight
        for k in range(8):
            if (2, 1, k) not in emitted_atoms:
                emit_out_atom(
                    2, 1, k, copy_eng=nc.scalar if k % 2 else nc.vector, deep_psum=True
                )
        # short HAM-warm bridge over the norm(7) reciprocal latency
        wacc2 = fillp.tile([128, 512], f32, tag="acc", name="warmacc2")
        for i in range(16):
            nc.tensor.matmul(
                wacc2[0:64, 0:256],
                lhsT=warm_sb[:, 0:64],
                rhs=x_sb[:, 0, 0:256],
                start=True,
                stop=True,
            )
        emit_norm_b_pe(7)
        # final unit's projection; per 128-row tile the two e-halves run as
        # FD=512 matmuls (fp32 PSUM out caps FD at one bank) with copies
        # alternating scalar/vector, and the y DMAs split across both rings;
        # accs alternate psen/fillp so they never wait on the same recycle
        for nt in range(12, 16):
            if nt % 2 == 0:
                ent = psen.tile([128, 2, 512], f32, tag="en", name=f"oacc7_{nt}")
                accs = [ent[:, 0, :], ent[:, 1, :]]
            else:
                accs = [
                    fillp.tile([128, 512], f32, tag="acc", name=f"oacc7_{nt}_{e}")
                    for e in (0, 1)
                ]
            ystage = stagep.tile([128, E], bf, tag="ystage", name=f"ystage7_{nt}")
            nsl_r = slice(nt * 128, (nt + 1) * 128)
            for ech in (0, 1):
                esl = slice(ech * 512, (ech + 1) * 512)
                nc.tensor.matmul(
                    accs[ech],
                    lhsT=outn_sb[:, 1, nsl_r],
                    rhs=wo_sb[:, 1, esl],
                    start=True,
                    stop=True,
                )
                if ech:
                    nc.scalar.copy(out=ystage[:, esl], in_=accs[ech])
                    nc.gpsimd.dma_start(out=y_d[1][nsl_r, esl], in_=ystage[:, esl])
                else:
                    nc.vector.tensor_copy(out=ystage[:, esl], in_=accs[ech])
                    nc.sync.dma_start(out=y_d[1][nsl_r, esl], in_=ystage[:, esl])


def _prep_core_inputs(x, Wq, bq, Wk, bk, Wv, bv, Wo, bo):
    """Build the 8 per-core input maps (host-side sharding + layout)."""
    xT_by_batch = []
    for b in range(2):
        xT = np.ascontiguousarray(x[b].T).astype(BF16).reshape(EC, 128, N)
        xa = np.ascontiguousarray(xT[:, :, 0:512])
        xb1 = np.ascontiguousarray(xT[:, :, 512:1024])
        xb2 = np.ascontiguousarray(xT[:, :, 1024:2048])
        xT_by_batch.append((xa, xb1, xb2))
    in_maps = []
    for c in range(8):
        b, g = divmod(c, 4)
        hsl = slice(g * HDL, (g + 1) * HDL)

        def pair_major(wT):
            # wT: [E, HDL] -> [128, 2, EC, 128] (partition-major, so every
            # DMA line is a contiguous 2KB run per partition)
            return np.ascontiguousarray(
                wT.reshape(EC, 128, 2, 128).transpose(1, 2, 0, 3)
            )

        wqT = pair_major((Wq[hsl, :] * SCALE).T.astype(BF16))
        wkT = pair_major(Wk[hsl, :].T.astype(BF16))
        wvT = np.ascontiguousarray(
            Wv[hsl, :].T.astype(BF16).reshape(EC, 128, HDL).transpose(1, 0, 2)
        )
        woT = np.ascontiguousarray(
            Wo[:, hsl].T.astype(BF16).reshape(2, 128, E).transpose(1, 0, 2)
        )
        bqs = np.ascontiguousarray(
            (bq[hsl] * SCALE).astype(np.float32).reshape(2, 128).T.reshape(128, 2, 1)
        )
        bks = np.ascontiguousarray(
            bk[hsl].astype(np.float32).reshape(2, 128).T.reshape(128, 2, 1)
        )
        in_maps.append(
            {
                "xTa": xT_by_batch[b][0],
                "xTb1": xT_by_batch[b][1],
                "xTb2": xT_by_batch[b][2],
                "wqT": wqT,
                "wkT": wkT,
                "wvT": wvT,
                "woT": woT,
                "bqs": bqs,
                "bks": bks,
            }
        )
    return in_maps


def run(inputs, trace=False, trace_kwargs=None):
    """Compile (cached), execute on 8 cores, gather.  Returns (y, results)."""
    from concourse.bass_utils import run_bass_kernel_spmd

    if "nc" not in _compiled:
        _compiled["nc"] = _build()
    nc = _compiled["nc"]

    in_maps = _prep_core_inputs(**inputs)
    kwargs = {}
    if trace:
        kwargs["trace"] = True
        kwargs["trace_kwargs"] = trace_kwargs or {}
    res = run_bass_kernel_spmd(nc, in_maps, core_ids=list(range(8)), **kwargs)

    x, Wo, bo, bv = inputs["x"], inputs["Wo"], inputs["bo"], inputs["bv"]
    y = np.zeros((2, N, E), np.float32)
    for c in range(8):
        b = c // 4
        y[b] += res.results[c]["y0"].astype(np.float32)
        y[b] += res.results[c]["y1"].astype(np.float32)
    y += (np.asarray(bv, np.float32) @ np.asarray(Wo, np.float32).T + np.asarray(bo, np.float32))[None, None, :]
    return y.astype(np.float32), res


def kernel(**inputs):
    inputs = {k: np.asarray(v) for k, v in inputs.items()}
    y, _ = run(inputs)
    return y

